# revision 50
# baseline (speedup 1.0000x reference)
"""GRU-cell-variant kernel for Trainium2, data-parallel over batch on 8 cores.

Reference (per batch row b, hidden size H=1024):
    gates = sigmoid(x @ W_ih + b_ih + h @ W_hh + b_hh)   # [B, 2H]
    z, r  = gates[:, :H], gates[:, H:]
    cand  = tanh(x @ W_c + b_c + r * (h @ W_hc + b_hc))
    out   = (1 - z) * h + z * cand

Design:
  - 8-way batch shard (1024 rows/core), weights replicated. No collectives.
  - Everything on-chip is computed TRANSPOSED: out.T[o, b]. Weight tiles
    [k, o] load naturally as the stationary operand, host-pre-transposed
    x.T / h.T serve as the moving operand, and all biases are per-partition
    (free bias-add on the ACT engine).
  - Mixed fp8/fp16 matmuls: the z/r gate matmuls, h@W_hc, and half of
    x@W_c's contraction run as fp8-e4m3 DoubleRow (2 contraction chunks
    per PE pass -> 2x the fp16 matmul rate, 216ns per K=256xN=512 pass
    measured); the other half of x@W_c stays fp16 because its
    quantization error feeds tanh unattenuated. Measured L2 error:
    all-fp8 2.05e-2 (fails), this mix 1.77e-2 vs the 2e-2 gate.
  - fp8 operands are pre-scaled on the host (x,h by 2^4; W by 2^8) to stay
    clear of e4m3 subnormals; the combined 2^-12 descale folds into the
    scale parameter of the existing sigmoid/tanh activation ops. The fp16
    W_c half is pre-scaled by 2^12 so all candidate partial sums share one
    scale. fp32 PSUM accumulation throughout; h-residual path in fp16.
  - All weights are SBUF-resident and streamed in once. DMA discipline
    (HWDGE serves in-flight DMAs round-robin per descriptor, and the Tile
    scheduler hoists DMA issues ahead of compute): (1) every cold-start-
    critical transfer rides ONE queue (sync) in strict need order with
    nothing else on it; (2) every bulk transfer (x16 b1, h16 j1+, the
    j>=1 weight train) is issued from the compute-free GPSIMD FIFO behind
    an explicit WAW gate -- a 1-column DVE/GPSIMD copy from a compute
    result into the DMA's destination -- so it cannot start before the
    pipeline actually needs it; (3) output stores ride the sync queue,
    idle after cold start. ACT runs activations only.
  - Elementwise blend runs in fp16 where PSUM isn't involved (2x DVE),
    output is stored fp16 and upcast on the host. The last tile computes
    its z-gate matmul LAST and blends in 256-wide halves so the serial
    post-matmul chain is ~2.7us instead of ~10us.
"""

import numpy as np
import ml_dtypes

import concourse.bass as bass
import concourse.mybir as mybir
import concourse.tile as tile
from concourse import bacc
from concourse.bass_utils import run_bass_kernel_spmd

N_CORES = 8
B = 8192
H = 1024
BL = B // N_CORES  # batch rows per core
P = 128
KC = H // P  # 8 contraction chunks of 128 per 1024-wide operand
NJ = H // P  # 8 hidden-dim tiles
NB = BL // 512  # 2 moving halves of 512 batch columns

F8 = mybir.dt.float8e4
F16 = mybir.dt.float16
F32 = mybir.dt.float32
AF = mybir.ActivationFunctionType
ALU = mybir.AluOpType
DR = mybir.MatmulPerfMode.DoubleRow

ASCALE = 16.0  # activation fp8 pre-scale
WSCALE = 256.0  # weight fp8 pre-scale
SCALE_INV = 1.0 / (ASCALE * WSCALE)  # descale folded into ACT ops

_CACHE = {}


def _build_program():
    nc = bacc.Bacc(
        "TRN2",
        target_bir_lowering=False,
        debug=False,
        enable_asserts=False,
        num_devices=N_CORES,
    )

    # DRAM inputs, already packed on the host into SBUF-friendly layouts.
    # x8/h8:  [p, hf*4096 + kc*512 + b] = x[hf*512 + b, kc*128 + p] * 16
    #         (fp8 e4m3, batch-half-major so every cold DMA is a contiguous
    #          per-partition run with >=2KB lines — small-line strided DMAs
    #          get starved by the round-robin descriptor service when bulk
    #          weight DMAs share the queue)
    # x16:    same layout, unscaled fp16 (W_c matmul operand)
    # h16:    [p, j*BL + b] = h[b, j*128 + p]  fp16 (residual path)
    # Wg:     [p, j*4096 + g*2048 + kc*128 + jj]
    #           = concat([W_ih, W_hh])[kc*128+p, g*1024 + j*128 + jj] * 256
    #         (fp8; g=0 -> z gate, g=1 -> r gate; per-j single contiguous DMA)
    # Whc:    [p, j*1024 + kc*128 + jj] = W_hc[kc*128+p, j*128+jj]*256  (fp8)
    # Wc:     same layout, W_c * 4096  (fp16)
    # bg:     [p, t] = (b_ih+b_hh)[t*128+p]; bc analogous; bhc pre-scaled 4096.
    x8 = nc.dram_tensor("x8", [P, KC * BL], F8, kind="ExternalInput").ap()
    h8 = nc.dram_tensor("h8", [P, KC * BL], F8, kind="ExternalInput").ap()
    x16 = nc.dram_tensor("x16", [P, 4 * BL], F16, kind="ExternalInput").ap()
    h16 = nc.dram_tensor("h16", [P, NJ * BL], F16, kind="ExternalInput").ap()
    Wg = nc.dram_tensor("Wg", [P, NJ * 4096], F8, kind="ExternalInput").ap()
    Wc = nc.dram_tensor("Wc", [P, NJ * 512], F16, kind="ExternalInput").ap()
    Wc8 = nc.dram_tensor("Wc8", [P, NJ * 512], F8, kind="ExternalInput").ap()
    Whc = nc.dram_tensor("Whc", [P, NJ * H], F8, kind="ExternalInput").ap()
    bg = nc.dram_tensor("bg", [P, 16], F32, kind="ExternalInput").ap()
    bc = nc.dram_tensor("bc", [P, NJ], F32, kind="ExternalInput").ap()
    bhc = nc.dram_tensor("bhc", [P, NJ], F32, kind="ExternalInput").ap()
    outT = nc.dram_tensor("outT", [P, NJ * BL], F16, kind="ExternalOutput").ap()

    with tile.TileContext(nc) as tc:
        with (
            tc.tile_pool(name="const", bufs=1) as cpool,
            tc.tile_pool(name="psum", bufs=8, space="PSUM") as ppool,
            tc.tile_pool(name="gates", bufs=6) as gpool,
            tc.tile_pool(name="work", bufs=12) as wpool,
        ):
            bg_sb = cpool.tile([P, 16], F32, tag="bg")
            bc_sb = cpool.tile([P, NJ], F32, tag="bc")
            bhc_sb = cpool.tile([P, NJ], F32, tag="bhc")

            # Resident activations and weights.
            x8_sb = cpool.tile([P, KC * BL], F8, tag="x8")
            h8_sb = cpool.tile([P, KC * BL], F8, tag="h8")
            x16_sb = cpool.tile([P, 4 * BL], F16, tag="x16")
            h16_sb = cpool.tile([P, NJ * BL], F16, tag="h16")
            wzr_sb = cpool.tile([P, NJ * 4096], F8, tag="wzr")
            whc_sb = cpool.tile([P, NJ * H], F8, tag="whc")
            wc_sb = cpool.tile([P, NJ * 512], F16, tag="wc")
            wc8_sb = cpool.tile([P, NJ * 512], F8, tag="wc8")

            def pair8(sb, hf, k0):
                # [p, 2, 512] DoubleRow rhs view of contraction pair (k0, k0+1)
                off = hf * 4096 + k0 * 512
                return sb[:, off : off + 1024].rearrange("p (k b) -> p k b", k=2)

            def wz3(j):
                return wzr_sb[:, j * 4096 : j * 4096 + 2048].rearrange(
                    "p (kc m) -> p kc m", kc=16
                )

            def wr3(j):
                return wzr_sb[:, j * 4096 + 2048 : (j + 1) * 4096].rearrange(
                    "p (kc m) -> p kc m", kc=16
                )

            def whc3(j):
                return whc_sb[:, j * H : (j + 1) * H].rearrange(
                    "p (kc m) -> p kc m", kc=KC
                )

            # ---- cold-start DMA issue trains ----
            # In-flight DMAs on one queue share descriptor-level round-robin
            # bandwidth, so each ring carries only same-criticality
            # transfers, every transfer is a contiguous per-partition run
            # (2-4KB lines), and the bulk j>=1 weight train is issued from
            # the ACT FIFO between sigmoids (compute-paced backpressure).
            # ACT ring cold: j0 weights + constants only. wz j0 is chunked
            # so the first matmul pairs start before the full tile lands.
            # ALL cold-start transfers ride ONE queue (sync) in strict need
            # order: with a single active queue there is no cross-queue
            # bandwidth competition, so the critical item is always among
            # the <=8 in-flight transfers and completes in need order.
            # (x16 b1, h16 j1 and the j>=1 weight train are issued from the
            # ACT FIFO after the first sigmoids — compute-paced.)
            # x-operands (consumed by the j0 x-phase) strictly before the
            # h-operands (h-phase) — arrival-matched to the matmul order
            nc.sync.dma_start(wzr_sb[:, 0:256], Wg[:, 0:256])  # wz j0 pair c0
            nc.sync.dma_start(x8_sb[:, 0:1024], x8[:, 0:1024])  # x8 b0 pair 0
            nc.sync.dma_start(wzr_sb[:, 256:1024], Wg[:, 256:1024])  # wz j0 lo
            nc.sync.dma_start(x8_sb[:, 1024:2048], x8[:, 1024:2048])  # x8 b0 pair 1
            nc.sync.dma_start(wzr_sb[:, 1024:2048], Wg[:, 1024:2048])  # wz j0 hi
            nc.sync.dma_start(x8_sb[:, 2048:4096], x8[:, 2048:4096])  # x8 b0 hi
            nc.sync.dma_start(wzr_sb[:, 2048:4096], Wg[:, 2048:4096])  # wr j0
            nc.sync.dma_start(x8_sb[:, 4096:8192], x8[:, 4096:8192])  # x8 b1
            nc.sync.dma_start(h8_sb[:, 0:4096], h8[:, 0:4096])  # h8 b0
            nc.sync.dma_start(bg_sb[:], bg[:])
            nc.sync.dma_start(h8_sb[:, 4096:8192], h8[:, 4096:8192])  # h8 b1
            nc.sync.dma_start(whc_sb[:, 0:H], Whc[:, 0:H])  # whc j0
            nc.sync.dma_start(wc8_sb[:, 0:512], Wc8[:, 0:512])  # wc8 j0
            nc.sync.dma_start(wc_sb[:, 0:512], Wc[:, 0:512])  # wc j0
            nc.sync.dma_start(x16_sb[:, 0:1024], x16[:, 0:1024])  # x16 b0 lo
            nc.sync.dma_start(x16_sb[:, 1024:2048], x16[:, 1024:2048])  # x16 b0 hi
            nc.sync.dma_start(bhc_sb[:], bhc[:])
            nc.sync.dma_start(bc_sb[:], bc[:])

            # PE warmup: ~90 dummy matmuls on scratch data bridge the HAM
            # clock-gate's 3.4us busy window during the cold DMA phase, so
            # real matmuls start at 2.4GHz instead of 1.2GHz. (Viable now
            # that the cold critical path is lean enough for arrivals to
            # keep up with warm-clock consumption.)
            warm_sb = cpool.tile([P, 64], F16, tag="warm")
            nc.vector.memset(warm_sb[:], 0.0)
            def gated_dma(sb, dram, a, b, gate_src):
                # The scheduler hoists DMA issues ahead of compute, so every
                # bulk transfer gets a real WAW hazard: a 1-column copy from
                # a compute result into the DMA's destination region. The
                # issue then cannot run before that compute finished. All on
                # the GPSIMD FIFO, which carries no latency-critical work.
                nc.gpsimd.tensor_copy(sb[:, a : a + 1], gate_src)
                nc.gpsimd.dma_start(sb[:, a:b], dram[:, a:b])

            def load_weights(j, gate_src):
                gated_dma(wzr_sb, Wg, j * 4096, (j + 1) * 4096, gate_src)
                gated_dma(whc_sb, Whc, j * H, (j + 1) * H, gate_src)
                gated_dma(wc_sb, Wc, j * 512, (j + 1) * 512, gate_src)
                gated_dma(wc8_sb, Wc8, j * 512, (j + 1) * 512, gate_src)

            def gate_matmuls(psum, w3, hf, cs=range(KC)):
                # accumulate over [x;h]: 8 DoubleRow passes of K=256 each;
                # pair c<4 reads x8, c>=4 reads h8
                for c in cs:
                    src = x8_sb if c < KC // 2 else h8_sb
                    nc.tensor.matmul(
                        psum[:],
                        lhsT=w3[:, 2 * c : 2 * c + 2, :],
                        rhs=pair8(src, hf, (2 * c) % KC),
                        start=(c == 0),
                        stop=(c == KC - 1),
                        perf_mode=DR,
                    )

            def hc_matmuls(psum, j, hf):
                w3 = whc3(j)
                for c in range(KC // 2):
                    nc.tensor.matmul(
                        psum[:],
                        lhsT=w3[:, 2 * c : 2 * c + 2, :],
                        rhs=pair8(h8_sb, hf, 2 * c),
                        start=(c == 0),
                        stop=(c == KC // 2 - 1),
                        perf_mode=DR,
                    )

            def c_matmuls(psum, j, hf):
                # contraction kc0-3 in fp8 DoubleRow (via resident x8),
                # kc4-7 in fp16 — error-budget split measured at 1.77e-2
                # total vs the 2e-2 gate
                w83 = wc8_sb[:, j * 512 : (j + 1) * 512].rearrange(
                    "p (kc m) -> p kc m", kc=4
                )
                for c in range(2):
                    nc.tensor.matmul(
                        psum[:],
                        lhsT=w83[:, 2 * c : 2 * c + 2, :],
                        rhs=pair8(x8_sb, hf, 2 * c),
                        start=(c == 0),
                        stop=False,
                        perf_mode=DR,
                    )
                for k in range(4):
                    nc.tensor.matmul(
                        psum[:],
                        lhsT=wc_sb[:, (j * 4 + k) * P : (j * 4 + k + 1) * P],
                        rhs=x16_sb[:, hf * 2048 + k * 512 : hf * 2048 + k * 512 + 512],
                        start=False,
                        stop=(k == 3),
                    )

            def act_z(j, b0, pz):
                # z = sigmoid(pz/4096 + bg_z); zh = (z-1)*h  (fp16, 2x DVE)
                z = gpool.tile([P, 512], F16, tag="g")
                nc.scalar.activation(
                    z[:], pz[:], AF.Sigmoid,
                    bias=bg_sb[:, j : j + 1], scale=SCALE_INV,
                )
                zh = wpool.tile([P, 512], F16, tag="w16")
                nc.vector.scalar_tensor_tensor(
                    zh[:], z[:], 1.0, h16_sb[:, j * BL + b0 : j * BL + b0 + 512],
                    ALU.subtract, ALU.mult,
                )
                return z, zh

            def act_r(j, pr):
                r = gpool.tile([P, 512], F32, tag="gr")
                nc.scalar.activation(
                    r[:], pr[:], AF.Sigmoid,
                    bias=bg_sb[:, NJ + j : NJ + j + 1], scale=SCALE_INV,
                )
                return r

            def blend(j, b0, z, zh, r, ph, px):
                # cand = tanh((px + (ph + bhc~)*r)/4096 + bc);
                # out = z*cand - (z-1)*h
                rh = wpool.tile([P, 512], F32, tag="w32")
                nc.vector.scalar_tensor_tensor(
                    rh[:], ph[:], bhc_sb[:, j : j + 1], r[:], ALU.add, ALU.mult
                )
                s = wpool.tile([P, 512], F32, tag="w32")
                nc.vector.tensor_add(s[:], px[:], rh[:])
                cand = wpool.tile([P, 512], F16, tag="w16")
                nc.scalar.activation(
                    cand[:], s[:], AF.Tanh, bias=bc_sb[:, j : j + 1], scale=SCALE_INV
                )
                m = wpool.tile([P, 512], F16, tag="w16")
                nc.vector.tensor_mul(m[:], z[:], cand[:])
                o = wpool.tile([P, 512], F16, tag="w16")
                nc.vector.tensor_sub(o[:], m[:], zh[:])
                nc.sync.dma_start(
                    outT[:, j * BL + b0 : j * BL + b0 + 512], o[:]
                )

            # ---- j = 0: gates for both b-halves first (their fp8 operands
            # arrive first), candidate x-matmuls last (x16 arrives later).
            # Uses all 8 PSUM banks.
            pz0 = ppool.tile([P, 512], F32, tag="ps")
            pr0 = ppool.tile([P, 512], F32, tag="ps")
            for _ in range(90):
                nc.tensor.matmul(
                    pz0[0:64, 0:64], lhsT=warm_sb[:], rhs=warm_sb[:],
                    start=True, stop=True,
                )
            # x-phase for all four j0 gate groups (operands: wz0, x8 b0,
            # wr0, x8 b1 — all arriving before any h8), then the h-phase.
            # Four open PSUM accumulation groups on distinct banks.
            pz1 = ppool.tile([P, 512], F32, tag="ps")
            pr1 = ppool.tile([P, 512], F32, tag="ps")
            gate_matmuls(pz0, wz3(0), 0, cs=range(0, 4))
            gate_matmuls(pr0, wr3(0), 0, cs=range(0, 4))
            gate_matmuls(pz1, wz3(0), 1, cs=range(0, 4))
            gate_matmuls(pr1, wr3(0), 1, cs=range(0, 4))
            gate_matmuls(pz0, wz3(0), 0, cs=range(4, 8))
            gate_matmuls(pr0, wr3(0), 0, cs=range(4, 8))
            gate_matmuls(pz1, wz3(0), 1, cs=range(4, 8))
            gate_matmuls(pr1, wr3(0), 1, cs=range(4, 8))
            # h16 j0 and the j1 weight train gate on the completed pz0 PSUM
            # via DVE copies (GPSIMD can't read PSUM; this fires ~1us before
            # the z0 sigmoid result exists)
            nc.vector.tensor_copy(h16_sb[:, 0:1], pz0[:, 0:1])
            nc.gpsimd.dma_start(h16_sb[:, 0:BL], h16[:, 0:BL])  # h16 j0
            for sb, dram, a, b in (
                (x16_sb, x16, 2048, 3072),  # x16 b1 lo
                (x16_sb, x16, 3072, 4096),  # x16 b1 hi
                (wzr_sb, Wg, 4096, 8192),
                (whc_sb, Whc, H, 2 * H),
                (wc_sb, Wc, 512, 1024),
                (wc8_sb, Wc8, 512, 1024),
            ):
                nc.vector.tensor_copy(sb[:, a : a + 1], pz0[:, 0:1])
                nc.gpsimd.dma_start(sb[:, a:b], dram[:, a:b])
            z0, zh0 = act_z(0, 0, pz0)
            zg0 = z0[:, 0:1]
            r0 = act_r(0, pr0)
            gated_dma(h16_sb, h16, BL, 2 * BL, zg0)  # h16 j1
            z1, zh1 = act_z(0, 512, pz1)
            r1 = act_r(0, pr1)
            ph0 = ppool.tile([P, 512], F32, tag="ps")
            hc_matmuls(ph0, 0, 0)
            ph1 = ppool.tile([P, 512], F32, tag="ps")
            hc_matmuls(ph1, 0, 1)
            px0 = ppool.tile([P, 512], F32, tag="ps")
            c_matmuls(px0, 0, 0)
            blend(0, 0, z0, zh0, r0, ph0, px0)
            px1 = ppool.tile([P, 512], F32, tag="ps")
            c_matmuls(px1, 0, 1)
            blend(0, 512, z1, zh1, r1, ph1, px1)

            # j2 stream gated on z1
            gated_dma(h16_sb, h16, 2 * BL, 3 * BL, z1[:, 0:1])  # h16 j2
            load_weights(2, z1[:, 0:1])

            # ---- steady state ----
            for j in range(1, NJ):
                for b in range(NB):
                    b0 = b * 512
                    if j == NJ - 1 and b == NB - 1:
                        break  # last tile handled below
                    pz = ppool.tile([P, 512], F32, tag="ps")
                    gate_matmuls(pz, wz3(j), b)
                    z, zh = act_z(j, b0, pz)
                    pr = ppool.tile([P, 512], F32, tag="ps")
                    gate_matmuls(pr, wr3(j), b)
                    r = act_r(j, pr)
                    if b == 0 and j + 2 < NJ:
                        gated_dma(
                            h16_sb, h16, (j + 2) * BL, (j + 3) * BL, z[:, 0:1]
                        )
                        load_weights(j + 2, z[:, 0:1])
                    ph = ppool.tile([P, 512], F32, tag="ps")
                    hc_matmuls(ph, j, b)
                    px = ppool.tile([P, 512], F32, tag="ps")
                    c_matmuls(px, j, b)
                    blend(j, b0, z, zh, r, ph, px)

            # ---- last tile: z-gate matmuls LAST, blend in 256-wide halves
            # so the post-matmul serial chain is short.
            j, b0 = NJ - 1, 512
            ph = ppool.tile([P, 512], F32, tag="ps")
            hc_matmuls(ph, j, 1)
            px = ppool.tile([P, 512], F32, tag="ps")
            c_matmuls(px, j, 1)
            pr = ppool.tile([P, 512], F32, tag="ps")
            gate_matmuls(pr, wr3(j), 1)
            pz = ppool.tile([P, 512], F32, tag="ps")
            gate_matmuls(pz, wz3(j), 1)
            r = act_r(j, pr)
            rh = wpool.tile([P, 512], F32, tag="w32")
            s = wpool.tile([P, 512], F32, tag="w32")
            cand = wpool.tile([P, 512], F16, tag="w16")
            z = gpool.tile([P, 512], F16, tag="g")
            zh = wpool.tile([P, 512], F16, tag="w16")
            m = wpool.tile([P, 512], F16, tag="w16")
            o = wpool.tile([P, 512], F16, tag="w16")
            H2 = 256
            for lo in (0, H2):
                nc.vector.scalar_tensor_tensor(
                    rh[:, lo : lo + H2], ph[:, lo : lo + H2],
                    bhc_sb[:, j : j + 1], r[:, lo : lo + H2], ALU.add, ALU.mult,
                )
            for lo in (0, H2):
                nc.vector.tensor_add(
                    s[:, lo : lo + H2], px[:, lo : lo + H2], rh[:, lo : lo + H2]
                )
            for lo in (0, H2):
                nc.scalar.activation(
                    cand[:, lo : lo + H2], s[:, lo : lo + H2], AF.Tanh,
                    bias=bc_sb[:, j : j + 1], scale=SCALE_INV,
                )
            for lo in (0, H2):
                nc.scalar.activation(
                    z[:, lo : lo + H2], pz[:, lo : lo + H2], AF.Sigmoid,
                    bias=bg_sb[:, j : j + 1], scale=SCALE_INV,
                )
            for lo in (0, H2):
                nc.vector.scalar_tensor_tensor(
                    zh[:, lo : lo + H2], z[:, lo : lo + H2], 1.0,
                    h16_sb[:, j * BL + b0 + lo : j * BL + b0 + lo + H2],
                    ALU.subtract, ALU.mult,
                )
            for lo in (0, H2):
                nc.vector.tensor_mul(
                    m[:, lo : lo + H2], z[:, lo : lo + H2], cand[:, lo : lo + H2]
                )
            for lo in (0, H2):
                nc.vector.tensor_sub(
                    o[:, lo : lo + H2], m[:, lo : lo + H2], zh[:, lo : lo + H2]
                )
            nc.sync.dma_start(outT[:, j * BL + b0 : j * BL + b0 + 512], o[:])

    nc.compile()
    return nc


def _pack_weights(W_ih, b_ih, W_hh, b_hh, W_c, b_c, W_hc, b_hc):
    f8 = ml_dtypes.float8_e4m3
    Wg_full = np.concatenate([W_ih, W_hh], axis=0)  # [2H, 2H] = [k, o]
    # [kc, p, g, j, jj] -> [p, j, g, kc, jj]
    WgH = np.ascontiguousarray(
        Wg_full.reshape(16, P, 2, NJ, P).transpose(1, 3, 2, 0, 4).reshape(P, NJ * 4096)
        * WSCALE
    ).astype(f8)
    WcH = np.ascontiguousarray(
        W_c[512:].reshape(4, P, NJ, P).transpose(1, 2, 0, 3).reshape(P, NJ * 512)
        * (ASCALE * WSCALE)
    ).astype(np.float16)
    Wc8H = np.ascontiguousarray(
        W_c[:512].reshape(4, P, NJ, P).transpose(1, 2, 0, 3).reshape(P, NJ * 512)
        * WSCALE
    ).astype(f8)
    WhcH = np.ascontiguousarray(
        W_hc.reshape(KC, P, NJ, P).transpose(1, 2, 0, 3).reshape(P, NJ * H) * WSCALE
    ).astype(f8)
    bgH = np.ascontiguousarray((b_ih + b_hh).reshape(16, P).T).astype(np.float32)
    bcH = np.ascontiguousarray(b_c.reshape(NJ, P).T).astype(np.float32)
    bhcH = np.ascontiguousarray(b_hc.reshape(NJ, P).T * (ASCALE * WSCALE)).astype(
        np.float32
    )
    return WgH, WcH, Wc8H, WhcH, bgH, bcH, bhcH


def _pack_acts(a, dtype, scale=1.0):
    # [BL, H] -> [p, hf*(KC*512) + kc*512 + b] with a[hf*512 + b, kc*128+p]
    out = a.T.reshape(KC, P, NB, 512).transpose(1, 2, 0, 3).reshape(P, KC * BL)
    if scale != 1.0:
        out = out * scale
    return np.ascontiguousarray(out).astype(dtype)


def _pack_x16(a):
    # kc4-7 only (the fp8-DoubleRow half of x@W_c reads x8 instead):
    # [p, hf*2048 + (kc-4)*512 + b]
    return np.ascontiguousarray(
        a.T[512:].reshape(4, P, NB, 512).transpose(1, 2, 0, 3).reshape(P, 4 * BL)
    ).astype(np.float16)


def _pack_h(a):
    # j-major residual layout: [p, j*BL + b] with a[b, j*128+p]
    return np.ascontiguousarray(
        a.T.reshape(NJ, P, BL).transpose(1, 0, 2).reshape(P, NJ * BL)
    ).astype(np.float16)


def kernel(input, hx, W_ih, b_ih, W_hh, b_hh, W_c, b_c, W_hc, b_hc):
    input = np.asarray(input, np.float32)
    hx = np.asarray(hx, np.float32)
    if "nc" not in _CACHE:
        _CACHE["nc"] = _build_program()
    nc = _CACHE["nc"]

    WgH, WcH, Wc8H, WhcH, bgH, bcH, bhcH = _pack_weights(
        np.asarray(W_ih, np.float32), np.asarray(b_ih, np.float32),
        np.asarray(W_hh, np.float32), np.asarray(b_hh, np.float32),
        np.asarray(W_c, np.float32), np.asarray(b_c, np.float32),
        np.asarray(W_hc, np.float32), np.asarray(b_hc, np.float32),
    )

    f8 = ml_dtypes.float8_e4m3
    in_maps = []
    for i in range(N_CORES):
        xs = input[i * BL : (i + 1) * BL]
        hs = hx[i * BL : (i + 1) * BL]
        in_maps.append(
            {
                "x8": _pack_acts(xs, f8, ASCALE),
                "h8": _pack_acts(hs, f8, ASCALE),
                "x16": _pack_x16(xs),
                "h16": _pack_h(hs),
                "Wg": WgH,
                "Wc": WcH,
                "Wc8": Wc8H,
                "Whc": WhcH,
                "bg": bgH,
                "bc": bcH,
                "bhc": bhcH,
            }
        )

    res = run_bass_kernel_spmd(nc, in_maps, core_ids=list(range(N_CORES)))
    out = np.empty((B, H), np.float32)
    for i, r in enumerate(res.results):
        o = (
            np.asarray(r["outT"], dtype=np.float32)
            .reshape(P, NJ, BL)
            .transpose(2, 1, 0)
            .reshape(BL, H)
        )
        out[i * BL : (i + 1) * BL] = o
    return out


# revision 51
# speedup vs baseline: 1.0002x; 1.0002x over previous
"""GRU-cell-variant kernel for Trainium2, data-parallel over batch on 8 cores.

Reference (per batch row b, hidden size H=1024):
    gates = sigmoid(x @ W_ih + b_ih + h @ W_hh + b_hh)   # [B, 2H]
    z, r  = gates[:, :H], gates[:, H:]
    cand  = tanh(x @ W_c + b_c + r * (h @ W_hc + b_hc))
    out   = (1 - z) * h + z * cand

Design:
  - 8-way batch shard (1024 rows/core), weights replicated. No collectives.
  - Everything on-chip is computed TRANSPOSED: out.T[o, b]. Weight tiles
    [k, o] load naturally as the stationary operand, host-pre-transposed
    x.T / h.T serve as the moving operand, and all biases are per-partition
    (free bias-add on the ACT engine).
  - Mixed fp8/fp16 matmuls: the z/r gate matmuls, h@W_hc, and half of
    x@W_c's contraction run as fp8-e4m3 DoubleRow (2 contraction chunks
    per PE pass -> 2x the fp16 matmul rate, 216ns per K=256xN=512 pass
    measured); the other half of x@W_c stays fp16 because its
    quantization error feeds tanh unattenuated. Measured L2 error:
    all-fp8 2.05e-2 (fails), this mix 1.77e-2 vs the 2e-2 gate.
  - fp8 operands are pre-scaled on the host (x,h by 2^4; W by 2^8) to stay
    clear of e4m3 subnormals; the combined 2^-12 descale folds into the
    scale parameter of the existing sigmoid/tanh activation ops. The fp16
    W_c half is pre-scaled by 2^12 so all candidate partial sums share one
    scale. fp32 PSUM accumulation throughout; h-residual path in fp16.
  - All weights are SBUF-resident and streamed in once. DMA discipline
    (HWDGE serves in-flight DMAs round-robin per descriptor, and the Tile
    scheduler hoists DMA issues ahead of compute): (1) every cold-start-
    critical transfer rides ONE queue (sync) in strict need order with
    nothing else on it; (2) every bulk transfer (x16 b1, h16 j1+, the
    j>=1 weight train) is issued from the compute-free GPSIMD FIFO behind
    an explicit WAW gate -- a 1-column DVE/GPSIMD copy from a compute
    result into the DMA's destination -- so it cannot start before the
    pipeline actually needs it; (3) output stores ride the sync queue,
    idle after cold start. ACT runs activations only.
  - Elementwise blend runs in fp16 where PSUM isn't involved (2x DVE),
    output is stored fp16 and upcast on the host. The last tile computes
    its z-gate matmul LAST and blends in 256-wide halves so the serial
    post-matmul chain is ~2.7us instead of ~10us.
"""

import numpy as np
import ml_dtypes

import concourse.bass as bass
import concourse.mybir as mybir
import concourse.tile as tile
from concourse import bacc
from concourse.bass_utils import run_bass_kernel_spmd

N_CORES = 8
B = 8192
H = 1024
BL = B // N_CORES  # batch rows per core
P = 128
KC = H // P  # 8 contraction chunks of 128 per 1024-wide operand
NJ = H // P  # 8 hidden-dim tiles
NB = BL // 512  # 2 moving halves of 512 batch columns

F8 = mybir.dt.float8e4
F16 = mybir.dt.float16
F32 = mybir.dt.float32
AF = mybir.ActivationFunctionType
ALU = mybir.AluOpType
DR = mybir.MatmulPerfMode.DoubleRow

ASCALE = 16.0  # activation fp8 pre-scale
WSCALE = 256.0  # weight fp8 pre-scale
SCALE_INV = 1.0 / (ASCALE * WSCALE)  # descale folded into ACT ops

_CACHE = {}


def _build_program():
    nc = bacc.Bacc(
        "TRN2",
        target_bir_lowering=False,
        debug=False,
        enable_asserts=False,
        num_devices=N_CORES,
    )

    # DRAM inputs, already packed on the host into SBUF-friendly layouts.
    # x8/h8:  [p, hf*4096 + kc*512 + b] = x[hf*512 + b, kc*128 + p] * 16
    #         (fp8 e4m3, batch-half-major so every cold DMA is a contiguous
    #          per-partition run with >=2KB lines — small-line strided DMAs
    #          get starved by the round-robin descriptor service when bulk
    #          weight DMAs share the queue)
    # x16:    same layout, unscaled fp16 (W_c matmul operand)
    # h16:    [p, j*BL + b] = h[b, j*128 + p]  fp16 (residual path)
    # Wg:     [p, j*4096 + g*2048 + kc*128 + jj]
    #           = concat([W_ih, W_hh])[kc*128+p, g*1024 + j*128 + jj] * 256
    #         (fp8; g=0 -> z gate, g=1 -> r gate; per-j single contiguous DMA)
    # Whc:    [p, j*1024 + kc*128 + jj] = W_hc[kc*128+p, j*128+jj]*256  (fp8)
    # Wc:     same layout, W_c * 4096  (fp16)
    # bg:     [p, t] = (b_ih+b_hh)[t*128+p]; bc analogous; bhc pre-scaled 4096.
    x8 = nc.dram_tensor("x8", [P, KC * BL], F8, kind="ExternalInput").ap()
    h8 = nc.dram_tensor("h8", [P, KC * BL], F8, kind="ExternalInput").ap()
    x16 = nc.dram_tensor("x16", [P, 4 * BL], F16, kind="ExternalInput").ap()
    h16 = nc.dram_tensor("h16", [P, NJ * BL], F16, kind="ExternalInput").ap()
    Wg = nc.dram_tensor("Wg", [P, NJ * 4096], F8, kind="ExternalInput").ap()
    Wc = nc.dram_tensor("Wc", [P, NJ * 512], F16, kind="ExternalInput").ap()
    Wc8 = nc.dram_tensor("Wc8", [P, NJ * 512], F8, kind="ExternalInput").ap()
    Whc = nc.dram_tensor("Whc", [P, NJ * H], F8, kind="ExternalInput").ap()
    bg = nc.dram_tensor("bg", [P, 16], F32, kind="ExternalInput").ap()
    bc = nc.dram_tensor("bc", [P, NJ], F32, kind="ExternalInput").ap()
    bhc = nc.dram_tensor("bhc", [P, NJ], F32, kind="ExternalInput").ap()
    outT = nc.dram_tensor("outT", [P, NJ * BL], F16, kind="ExternalOutput").ap()

    with tile.TileContext(nc) as tc:
        with (
            tc.tile_pool(name="const", bufs=1) as cpool,
            tc.tile_pool(name="psum", bufs=8, space="PSUM") as ppool,
            tc.tile_pool(name="gates", bufs=6) as gpool,
            tc.tile_pool(name="work", bufs=12) as wpool,
        ):
            bg_sb = cpool.tile([P, 16], F32, tag="bg")
            bc_sb = cpool.tile([P, NJ], F32, tag="bc")
            bhc_sb = cpool.tile([P, NJ], F32, tag="bhc")

            # Resident activations and weights.
            x8_sb = cpool.tile([P, KC * BL], F8, tag="x8")
            h8_sb = cpool.tile([P, KC * BL], F8, tag="h8")
            x16_sb = cpool.tile([P, 4 * BL], F16, tag="x16")
            h16_sb = cpool.tile([P, NJ * BL], F16, tag="h16")
            wzr_sb = cpool.tile([P, NJ * 4096], F8, tag="wzr")
            whc_sb = cpool.tile([P, NJ * H], F8, tag="whc")
            wc_sb = cpool.tile([P, NJ * 512], F16, tag="wc")
            wc8_sb = cpool.tile([P, NJ * 512], F8, tag="wc8")

            def pair8(sb, hf, k0):
                # [p, 2, 512] DoubleRow rhs view of contraction pair (k0, k0+1)
                off = hf * 4096 + k0 * 512
                return sb[:, off : off + 1024].rearrange("p (k b) -> p k b", k=2)

            def wz3(j):
                return wzr_sb[:, j * 4096 : j * 4096 + 2048].rearrange(
                    "p (kc m) -> p kc m", kc=16
                )

            def wr3(j):
                return wzr_sb[:, j * 4096 + 2048 : (j + 1) * 4096].rearrange(
                    "p (kc m) -> p kc m", kc=16
                )

            def whc3(j):
                return whc_sb[:, j * H : (j + 1) * H].rearrange(
                    "p (kc m) -> p kc m", kc=KC
                )

            # ---- cold-start DMA issue trains ----
            # In-flight DMAs on one queue share descriptor-level round-robin
            # bandwidth, so each ring carries only same-criticality
            # transfers, every transfer is a contiguous per-partition run
            # (2-4KB lines), and the bulk j>=1 weight train is issued from
            # the ACT FIFO between sigmoids (compute-paced backpressure).
            # ACT ring cold: j0 weights + constants only. wz j0 is chunked
            # so the first matmul pairs start before the full tile lands.
            # ALL cold-start transfers ride ONE queue (sync) in strict need
            # order: with a single active queue there is no cross-queue
            # bandwidth competition, so the critical item is always among
            # the <=8 in-flight transfers and completes in need order.
            # (x16 b1, h16 j1 and the j>=1 weight train are issued from the
            # ACT FIFO after the first sigmoids — compute-paced.)
            # x-operands (consumed by the j0 x-phase) strictly before the
            # h-operands (h-phase) — arrival-matched to the matmul order
            nc.sync.dma_start(wzr_sb[:, 0:256], Wg[:, 0:256])  # wz j0 pair c0
            nc.sync.dma_start(x8_sb[:, 0:1024], x8[:, 0:1024])  # x8 b0 pair 0
            nc.sync.dma_start(wzr_sb[:, 256:1024], Wg[:, 256:1024])  # wz j0 lo
            nc.sync.dma_start(x8_sb[:, 1024:2048], x8[:, 1024:2048])  # x8 b0 pair 1
            nc.sync.dma_start(wzr_sb[:, 1024:2048], Wg[:, 1024:2048])  # wz j0 hi
            nc.sync.dma_start(x8_sb[:, 2048:4096], x8[:, 2048:4096])  # x8 b0 hi
            nc.sync.dma_start(wzr_sb[:, 2048:4096], Wg[:, 2048:4096])  # wr j0
            nc.sync.dma_start(h8_sb[:, 0:4096], h8[:, 0:4096])  # h8 b0
            nc.sync.dma_start(bg_sb[:], bg[:])
            nc.sync.dma_start(x8_sb[:, 4096:8192], x8[:, 4096:8192])  # x8 b1
            nc.sync.dma_start(h8_sb[:, 4096:8192], h8[:, 4096:8192])  # h8 b1
            nc.sync.dma_start(whc_sb[:, 0:H], Whc[:, 0:H])  # whc j0
            nc.sync.dma_start(wc8_sb[:, 0:512], Wc8[:, 0:512])  # wc8 j0
            nc.sync.dma_start(wc_sb[:, 0:512], Wc[:, 0:512])  # wc j0
            nc.sync.dma_start(x16_sb[:, 0:1024], x16[:, 0:1024])  # x16 b0 lo
            nc.sync.dma_start(x16_sb[:, 1024:2048], x16[:, 1024:2048])  # x16 b0 hi
            nc.sync.dma_start(bhc_sb[:], bhc[:])
            nc.sync.dma_start(bc_sb[:], bc[:])

            def gated_dma(sb, dram, a, b, gate_src):
                # The scheduler hoists DMA issues ahead of compute, so every
                # bulk transfer gets a real WAW hazard: a 1-column copy from
                # a compute result into the DMA's destination region. The
                # issue then cannot run before that compute finished. All on
                # the GPSIMD FIFO, which carries no latency-critical work.
                nc.gpsimd.tensor_copy(sb[:, a : a + 1], gate_src)
                nc.gpsimd.dma_start(sb[:, a:b], dram[:, a:b])

            def load_weights(j, gate_src):
                gated_dma(wzr_sb, Wg, j * 4096, (j + 1) * 4096, gate_src)
                gated_dma(whc_sb, Whc, j * H, (j + 1) * H, gate_src)
                gated_dma(wc_sb, Wc, j * 512, (j + 1) * 512, gate_src)
                gated_dma(wc8_sb, Wc8, j * 512, (j + 1) * 512, gate_src)

            def gate_matmuls(psum, w3, hf, cs=range(KC)):
                # accumulate over [x;h]: 8 DoubleRow passes of K=256 each;
                # pair c<4 reads x8, c>=4 reads h8
                for c in cs:
                    src = x8_sb if c < KC // 2 else h8_sb
                    nc.tensor.matmul(
                        psum[:],
                        lhsT=w3[:, 2 * c : 2 * c + 2, :],
                        rhs=pair8(src, hf, (2 * c) % KC),
                        start=(c == 0),
                        stop=(c == KC - 1),
                        perf_mode=DR,
                    )

            def hc_matmuls(psum, j, hf):
                w3 = whc3(j)
                for c in range(KC // 2):
                    nc.tensor.matmul(
                        psum[:],
                        lhsT=w3[:, 2 * c : 2 * c + 2, :],
                        rhs=pair8(h8_sb, hf, 2 * c),
                        start=(c == 0),
                        stop=(c == KC // 2 - 1),
                        perf_mode=DR,
                    )

            def c_matmuls(psum, j, hf):
                # contraction kc0-3 in fp8 DoubleRow (via resident x8),
                # kc4-7 in fp16 — error-budget split measured at 1.77e-2
                # total vs the 2e-2 gate
                w83 = wc8_sb[:, j * 512 : (j + 1) * 512].rearrange(
                    "p (kc m) -> p kc m", kc=4
                )
                for c in range(2):
                    nc.tensor.matmul(
                        psum[:],
                        lhsT=w83[:, 2 * c : 2 * c + 2, :],
                        rhs=pair8(x8_sb, hf, 2 * c),
                        start=(c == 0),
                        stop=False,
                        perf_mode=DR,
                    )
                for k in range(4):
                    nc.tensor.matmul(
                        psum[:],
                        lhsT=wc_sb[:, (j * 4 + k) * P : (j * 4 + k + 1) * P],
                        rhs=x16_sb[:, hf * 2048 + k * 512 : hf * 2048 + k * 512 + 512],
                        start=False,
                        stop=(k == 3),
                    )

            def act_z(j, b0, pz):
                # z = sigmoid(pz/4096 + bg_z); zh = (z-1)*h  (fp16, 2x DVE)
                z = gpool.tile([P, 512], F16, tag="g")
                nc.scalar.activation(
                    z[:], pz[:], AF.Sigmoid,
                    bias=bg_sb[:, j : j + 1], scale=SCALE_INV,
                )
                zh = wpool.tile([P, 512], F16, tag="w16")
                nc.vector.scalar_tensor_tensor(
                    zh[:], z[:], 1.0, h16_sb[:, j * BL + b0 : j * BL + b0 + 512],
                    ALU.subtract, ALU.mult,
                )
                return z, zh

            def act_r(j, pr):
                r = gpool.tile([P, 512], F32, tag="gr")
                nc.scalar.activation(
                    r[:], pr[:], AF.Sigmoid,
                    bias=bg_sb[:, NJ + j : NJ + j + 1], scale=SCALE_INV,
                )
                return r

            def blend(j, b0, z, zh, r, ph, px):
                # cand = tanh((px + (ph + bhc~)*r)/4096 + bc);
                # out = z*cand - (z-1)*h
                rh = wpool.tile([P, 512], F32, tag="w32")
                nc.vector.scalar_tensor_tensor(
                    rh[:], ph[:], bhc_sb[:, j : j + 1], r[:], ALU.add, ALU.mult
                )
                s = wpool.tile([P, 512], F32, tag="w32")
                nc.vector.tensor_add(s[:], px[:], rh[:])
                cand = wpool.tile([P, 512], F16, tag="w16")
                nc.scalar.activation(
                    cand[:], s[:], AF.Tanh, bias=bc_sb[:, j : j + 1], scale=SCALE_INV
                )
                m = wpool.tile([P, 512], F16, tag="w16")
                nc.vector.tensor_mul(m[:], z[:], cand[:])
                o = wpool.tile([P, 512], F16, tag="w16")
                nc.vector.tensor_sub(o[:], m[:], zh[:])
                nc.sync.dma_start(
                    outT[:, j * BL + b0 : j * BL + b0 + 512], o[:]
                )

            # ---- j = 0: gates for both b-halves first (their fp8 operands
            # arrive first), candidate x-matmuls last (x16 arrives later).
            # Uses all 8 PSUM banks.
            pz0 = ppool.tile([P, 512], F32, tag="ps")
            pr0 = ppool.tile([P, 512], F32, tag="ps")
            # interleave z/r with DMA arrival: wz0, x8 b0, wr0, h8 b0
            gate_matmuls(pz0, wz3(0), 0, cs=range(0, 4))
            gate_matmuls(pr0, wr3(0), 0, cs=range(0, 4))
            gate_matmuls(pz0, wz3(0), 0, cs=range(4, 8))
            gate_matmuls(pr0, wr3(0), 0, cs=range(4, 8))
            # h16 j0 and the j1 weight train gate on the completed pz0 PSUM
            # via DVE copies (GPSIMD can't read PSUM; this fires ~1us before
            # the z0 sigmoid result exists)
            nc.vector.tensor_copy(h16_sb[:, 0:1], pz0[:, 0:1])
            nc.gpsimd.dma_start(h16_sb[:, 0:BL], h16[:, 0:BL])  # h16 j0
            for sb, dram, a, b in (
                (x16_sb, x16, 2048, 3072),  # x16 b1 lo
                (x16_sb, x16, 3072, 4096),  # x16 b1 hi
                (wzr_sb, Wg, 4096, 8192),
                (whc_sb, Whc, H, 2 * H),
                (wc_sb, Wc, 512, 1024),
                (wc8_sb, Wc8, 512, 1024),
            ):
                nc.vector.tensor_copy(sb[:, a : a + 1], pz0[:, 0:1])
                nc.gpsimd.dma_start(sb[:, a:b], dram[:, a:b])
            z0, zh0 = act_z(0, 0, pz0)
            zg0 = z0[:, 0:1]
            r0 = act_r(0, pr0)
            gated_dma(h16_sb, h16, BL, 2 * BL, zg0)  # h16 j1
            pz1 = ppool.tile([P, 512], F32, tag="ps")
            gate_matmuls(pz1, wz3(0), 1, cs=range(0, 4))
            pr1 = ppool.tile([P, 512], F32, tag="ps")
            gate_matmuls(pr1, wr3(0), 1, cs=range(0, 4))
            gate_matmuls(pz1, wz3(0), 1, cs=range(4, 8))
            gate_matmuls(pr1, wr3(0), 1, cs=range(4, 8))
            z1, zh1 = act_z(0, 512, pz1)
            r1 = act_r(0, pr1)
            ph0 = ppool.tile([P, 512], F32, tag="ps")
            hc_matmuls(ph0, 0, 0)
            ph1 = ppool.tile([P, 512], F32, tag="ps")
            hc_matmuls(ph1, 0, 1)
            px0 = ppool.tile([P, 512], F32, tag="ps")
            c_matmuls(px0, 0, 0)
            blend(0, 0, z0, zh0, r0, ph0, px0)
            px1 = ppool.tile([P, 512], F32, tag="ps")
            c_matmuls(px1, 0, 1)
            blend(0, 512, z1, zh1, r1, ph1, px1)

            # j2 stream gated on z1
            gated_dma(h16_sb, h16, 2 * BL, 3 * BL, z1[:, 0:1])  # h16 j2
            load_weights(2, z1[:, 0:1])

            # ---- steady state ----
            for j in range(1, NJ):
                for b in range(NB):
                    b0 = b * 512
                    if j == NJ - 1 and b == NB - 1:
                        break  # last tile handled below
                    pz = ppool.tile([P, 512], F32, tag="ps")
                    gate_matmuls(pz, wz3(j), b)
                    z, zh = act_z(j, b0, pz)
                    pr = ppool.tile([P, 512], F32, tag="ps")
                    gate_matmuls(pr, wr3(j), b)
                    r = act_r(j, pr)
                    if b == 0 and j + 2 < NJ:
                        gated_dma(
                            h16_sb, h16, (j + 2) * BL, (j + 3) * BL, z[:, 0:1]
                        )
                        load_weights(j + 2, z[:, 0:1])
                    ph = ppool.tile([P, 512], F32, tag="ps")
                    hc_matmuls(ph, j, b)
                    px = ppool.tile([P, 512], F32, tag="ps")
                    c_matmuls(px, j, b)
                    blend(j, b0, z, zh, r, ph, px)

            # ---- last tile: z-gate matmuls LAST, blend in 256-wide halves
            # so the post-matmul serial chain is short.
            j, b0 = NJ - 1, 512
            ph = ppool.tile([P, 512], F32, tag="ps")
            hc_matmuls(ph, j, 1)
            px = ppool.tile([P, 512], F32, tag="ps")
            c_matmuls(px, j, 1)
            pr = ppool.tile([P, 512], F32, tag="ps")
            gate_matmuls(pr, wr3(j), 1)
            pz = ppool.tile([P, 512], F32, tag="ps")
            gate_matmuls(pz, wz3(j), 1)
            r = act_r(j, pr)
            rh = wpool.tile([P, 512], F32, tag="w32")
            s = wpool.tile([P, 512], F32, tag="w32")
            cand = wpool.tile([P, 512], F16, tag="w16")
            z = gpool.tile([P, 512], F16, tag="g")
            zh = wpool.tile([P, 512], F16, tag="w16")
            m = wpool.tile([P, 512], F16, tag="w16")
            o = wpool.tile([P, 512], F16, tag="w16")
            H2 = 256
            for lo in (0, H2):
                nc.vector.scalar_tensor_tensor(
                    rh[:, lo : lo + H2], ph[:, lo : lo + H2],
                    bhc_sb[:, j : j + 1], r[:, lo : lo + H2], ALU.add, ALU.mult,
                )
            for lo in (0, H2):
                nc.vector.tensor_add(
                    s[:, lo : lo + H2], px[:, lo : lo + H2], rh[:, lo : lo + H2]
                )
            for lo in (0, H2):
                nc.scalar.activation(
                    cand[:, lo : lo + H2], s[:, lo : lo + H2], AF.Tanh,
                    bias=bc_sb[:, j : j + 1], scale=SCALE_INV,
                )
            for lo in (0, H2):
                nc.scalar.activation(
                    z[:, lo : lo + H2], pz[:, lo : lo + H2], AF.Sigmoid,
                    bias=bg_sb[:, j : j + 1], scale=SCALE_INV,
                )
            for lo in (0, H2):
                nc.vector.scalar_tensor_tensor(
                    zh[:, lo : lo + H2], z[:, lo : lo + H2], 1.0,
                    h16_sb[:, j * BL + b0 + lo : j * BL + b0 + lo + H2],
                    ALU.subtract, ALU.mult,
                )
            for lo in (0, H2):
                nc.vector.tensor_mul(
                    m[:, lo : lo + H2], z[:, lo : lo + H2], cand[:, lo : lo + H2]
                )
            for lo in (0, H2):
                nc.vector.tensor_sub(
                    o[:, lo : lo + H2], m[:, lo : lo + H2], zh[:, lo : lo + H2]
                )
            nc.sync.dma_start(outT[:, j * BL + b0 : j * BL + b0 + 512], o[:])

    nc.compile()
    return nc


def _pack_weights(W_ih, b_ih, W_hh, b_hh, W_c, b_c, W_hc, b_hc):
    f8 = ml_dtypes.float8_e4m3
    Wg_full = np.concatenate([W_ih, W_hh], axis=0)  # [2H, 2H] = [k, o]
    # [kc, p, g, j, jj] -> [p, j, g, kc, jj]
    WgH = np.ascontiguousarray(
        Wg_full.reshape(16, P, 2, NJ, P).transpose(1, 3, 2, 0, 4).reshape(P, NJ * 4096)
        * WSCALE
    ).astype(f8)
    WcH = np.ascontiguousarray(
        W_c[512:].reshape(4, P, NJ, P).transpose(1, 2, 0, 3).reshape(P, NJ * 512)
        * (ASCALE * WSCALE)
    ).astype(np.float16)
    Wc8H = np.ascontiguousarray(
        W_c[:512].reshape(4, P, NJ, P).transpose(1, 2, 0, 3).reshape(P, NJ * 512)
        * WSCALE
    ).astype(f8)
    WhcH = np.ascontiguousarray(
        W_hc.reshape(KC, P, NJ, P).transpose(1, 2, 0, 3).reshape(P, NJ * H) * WSCALE
    ).astype(f8)
    bgH = np.ascontiguousarray((b_ih + b_hh).reshape(16, P).T).astype(np.float32)
    bcH = np.ascontiguousarray(b_c.reshape(NJ, P).T).astype(np.float32)
    bhcH = np.ascontiguousarray(b_hc.reshape(NJ, P).T * (ASCALE * WSCALE)).astype(
        np.float32
    )
    return WgH, WcH, Wc8H, WhcH, bgH, bcH, bhcH


def _pack_acts(a, dtype, scale=1.0):
    # [BL, H] -> [p, hf*(KC*512) + kc*512 + b] with a[hf*512 + b, kc*128+p]
    out = a.T.reshape(KC, P, NB, 512).transpose(1, 2, 0, 3).reshape(P, KC * BL)
    if scale != 1.0:
        out = out * scale
    return np.ascontiguousarray(out).astype(dtype)


def _pack_x16(a):
    # kc4-7 only (the fp8-DoubleRow half of x@W_c reads x8 instead):
    # [p, hf*2048 + (kc-4)*512 + b]
    return np.ascontiguousarray(
        a.T[512:].reshape(4, P, NB, 512).transpose(1, 2, 0, 3).reshape(P, 4 * BL)
    ).astype(np.float16)


def _pack_h(a):
    # j-major residual layout: [p, j*BL + b] with a[b, j*128+p]
    return np.ascontiguousarray(
        a.T.reshape(NJ, P, BL).transpose(1, 0, 2).reshape(P, NJ * BL)
    ).astype(np.float16)


def kernel(input, hx, W_ih, b_ih, W_hh, b_hh, W_c, b_c, W_hc, b_hc):
    input = np.asarray(input, np.float32)
    hx = np.asarray(hx, np.float32)
    if "nc" not in _CACHE:
        _CACHE["nc"] = _build_program()
    nc = _CACHE["nc"]

    WgH, WcH, Wc8H, WhcH, bgH, bcH, bhcH = _pack_weights(
        np.asarray(W_ih, np.float32), np.asarray(b_ih, np.float32),
        np.asarray(W_hh, np.float32), np.asarray(b_hh, np.float32),
        np.asarray(W_c, np.float32), np.asarray(b_c, np.float32),
        np.asarray(W_hc, np.float32), np.asarray(b_hc, np.float32),
    )

    f8 = ml_dtypes.float8_e4m3
    in_maps = []
    for i in range(N_CORES):
        xs = input[i * BL : (i + 1) * BL]
        hs = hx[i * BL : (i + 1) * BL]
        in_maps.append(
            {
                "x8": _pack_acts(xs, f8, ASCALE),
                "h8": _pack_acts(hs, f8, ASCALE),
                "x16": _pack_x16(xs),
                "h16": _pack_h(hs),
                "Wg": WgH,
                "Wc": WcH,
                "Wc8": Wc8H,
                "Whc": WhcH,
                "bg": bgH,
                "bc": bcH,
                "bhc": bhcH,
            }
        )

    res = run_bass_kernel_spmd(nc, in_maps, core_ids=list(range(N_CORES)))
    out = np.empty((B, H), np.float32)
    for i, r in enumerate(res.results):
        o = (
            np.asarray(r["outT"], dtype=np.float32)
            .reshape(P, NJ, BL)
            .transpose(2, 1, 0)
            .reshape(BL, H)
        )
        out[i * BL : (i + 1) * BL] = o
    return out


# revision 52
# speedup vs baseline: 1.0356x; 1.0354x over previous
"""GRU-cell-variant kernel for Trainium2, data-parallel over batch on 8 cores.

Reference (per batch row b, hidden size H=1024):
    gates = sigmoid(x @ W_ih + b_ih + h @ W_hh + b_hh)   # [B, 2H]
    z, r  = gates[:, :H], gates[:, H:]
    cand  = tanh(x @ W_c + b_c + r * (h @ W_hc + b_hc))
    out   = (1 - z) * h + z * cand

Design:
  - 8-way batch shard (1024 rows/core), weights replicated. No collectives.
  - Everything on-chip is computed TRANSPOSED: out.T[o, b]. Weight tiles
    [k, o] load naturally as the stationary operand, host-pre-transposed
    x.T / h.T serve as the moving operand, and all biases are per-partition
    (free bias-add on the ACT engine).
  - Mixed fp8/fp16 matmuls: the z/r gate matmuls, h@W_hc, and half of
    x@W_c's contraction run as fp8-e4m3 DoubleRow (2 contraction chunks
    per PE pass -> 2x the fp16 matmul rate, 216ns per K=256xN=512 pass
    measured); the other half of x@W_c stays fp16 because its
    quantization error feeds tanh unattenuated. Measured L2 error:
    all-fp8 2.05e-2 (fails), this mix 1.77e-2 vs the 2e-2 gate.
  - fp8 operands are pre-scaled on the host (x,h by 2^4; W by 2^8) to stay
    clear of e4m3 subnormals; the combined 2^-12 descale folds into the
    scale parameter of the existing sigmoid/tanh activation ops. The fp16
    W_c half is pre-scaled by 2^12 so all candidate partial sums share one
    scale. fp32 PSUM accumulation throughout; h-residual path in fp16.
  - All weights are SBUF-resident and streamed in once. DMA discipline
    (HWDGE serves in-flight DMAs round-robin per descriptor, and the Tile
    scheduler hoists DMA issues ahead of compute): (1) every cold-start-
    critical transfer rides ONE queue (sync) in strict need order with
    nothing else on it; (2) every bulk transfer (x16 b1, h16 j1+, the
    j>=1 weight train) is issued from the compute-free GPSIMD FIFO behind
    an explicit WAW gate -- a 1-column DVE/GPSIMD copy from a compute
    result into the DMA's destination -- so it cannot start before the
    pipeline actually needs it; (3) output stores ride the sync queue,
    idle after cold start. ACT runs activations only.
  - Elementwise blend runs in fp16 where PSUM isn't involved (2x DVE),
    output is stored fp16 and upcast on the host. The last tile computes
    its z-gate matmul LAST and blends in 256-wide halves so the serial
    post-matmul chain is ~2.7us instead of ~10us.
"""

import numpy as np
import ml_dtypes

import concourse.bass as bass
import concourse.mybir as mybir
import concourse.tile as tile
from concourse import bacc
from concourse.bass_utils import run_bass_kernel_spmd

N_CORES = 8
B = 8192
H = 1024
BL = B // N_CORES  # batch rows per core
P = 128
KC = H // P  # 8 contraction chunks of 128 per 1024-wide operand
NJ = H // P  # 8 hidden-dim tiles
NB = BL // 512  # 2 moving halves of 512 batch columns

F8 = mybir.dt.float8e4
F16 = mybir.dt.float16
F32 = mybir.dt.float32
AF = mybir.ActivationFunctionType
ALU = mybir.AluOpType
DR = mybir.MatmulPerfMode.DoubleRow

ASCALE = 16.0  # activation fp8 pre-scale
WSCALE = 256.0  # weight fp8 pre-scale
SCALE_INV = 1.0 / (ASCALE * WSCALE)  # descale folded into ACT ops

_CACHE = {}


def _build_program():
    nc = bacc.Bacc(
        "TRN2",
        target_bir_lowering=False,
        debug=False,
        enable_asserts=False,
        num_devices=N_CORES,
    )

    # DRAM inputs, already packed on the host into SBUF-friendly layouts.
    # x8/h8:  [p, hf*4096 + kc*512 + b] = x[hf*512 + b, kc*128 + p] * 16
    #         (fp8 e4m3, batch-half-major so every cold DMA is a contiguous
    #          per-partition run with >=2KB lines — small-line strided DMAs
    #          get starved by the round-robin descriptor service when bulk
    #          weight DMAs share the queue)
    # x16:    same layout, unscaled fp16 (W_c matmul operand)
    # h16:    [p, j*BL + b] = h[b, j*128 + p]  fp16 (residual path)
    # Wg:     [p, j*4096 + g*2048 + kc*128 + jj]
    #           = concat([W_ih, W_hh])[kc*128+p, g*1024 + j*128 + jj] * 256
    #         (fp8; g=0 -> z gate, g=1 -> r gate; per-j single contiguous DMA)
    # Whc:    [p, j*1024 + kc*128 + jj] = W_hc[kc*128+p, j*128+jj]*256  (fp8)
    # Wc:     same layout, W_c * 4096  (fp16)
    # bg:     [p, t] = (b_ih+b_hh)[t*128+p]; bc analogous; bhc pre-scaled 4096.
    x8 = nc.dram_tensor("x8", [P, KC * BL], F8, kind="ExternalInput").ap()
    h8 = nc.dram_tensor("h8", [P, KC * BL], F8, kind="ExternalInput").ap()
    x16 = nc.dram_tensor("x16", [P, 4 * BL], F16, kind="ExternalInput").ap()
    h16 = nc.dram_tensor("h16", [P, NJ * BL], F16, kind="ExternalInput").ap()
    Wg = nc.dram_tensor("Wg", [P, NJ * 4096], F8, kind="ExternalInput").ap()
    Wc = nc.dram_tensor("Wc", [P, NJ * 512], F16, kind="ExternalInput").ap()
    Wc8 = nc.dram_tensor("Wc8", [P, NJ * 512], F8, kind="ExternalInput").ap()
    Whc = nc.dram_tensor("Whc", [P, NJ * H], F8, kind="ExternalInput").ap()
    bg = nc.dram_tensor("bg", [P, 16], F32, kind="ExternalInput").ap()
    bc = nc.dram_tensor("bc", [P, NJ], F32, kind="ExternalInput").ap()
    bhc = nc.dram_tensor("bhc", [P, NJ], F32, kind="ExternalInput").ap()
    outT = nc.dram_tensor("outT", [P, NJ * BL], F16, kind="ExternalOutput").ap()

    with tile.TileContext(nc) as tc:
        with (
            tc.tile_pool(name="const", bufs=1) as cpool,
            tc.tile_pool(name="psum", bufs=8, space="PSUM") as ppool,
            tc.tile_pool(name="gates", bufs=6) as gpool,
            tc.tile_pool(name="work", bufs=12) as wpool,
        ):
            bg_sb = cpool.tile([P, 16], F32, tag="bg")
            bc_sb = cpool.tile([P, NJ], F32, tag="bc")
            bhc_sb = cpool.tile([P, NJ], F32, tag="bhc")

            # Resident activations and weights.
            x8_sb = cpool.tile([P, KC * BL], F8, tag="x8")
            h8_sb = cpool.tile([P, KC * BL], F8, tag="h8")
            x16_sb = cpool.tile([P, 4 * BL], F16, tag="x16")
            h16_sb = cpool.tile([P, NJ * BL], F16, tag="h16")
            wzr_sb = cpool.tile([P, NJ * 4096], F8, tag="wzr")
            whc_sb = cpool.tile([P, NJ * H], F8, tag="whc")
            wc_sb = cpool.tile([P, NJ * 512], F16, tag="wc")
            wc8_sb = cpool.tile([P, NJ * 512], F8, tag="wc8")

            def pair8(sb, hf, k0):
                # [p, 2, 512] DoubleRow rhs view of contraction pair (k0, k0+1)
                off = hf * 4096 + k0 * 512
                return sb[:, off : off + 1024].rearrange("p (k b) -> p k b", k=2)

            def wz3(j):
                return wzr_sb[:, j * 4096 : j * 4096 + 2048].rearrange(
                    "p (kc m) -> p kc m", kc=16
                )

            def wr3(j):
                return wzr_sb[:, j * 4096 + 2048 : (j + 1) * 4096].rearrange(
                    "p (kc m) -> p kc m", kc=16
                )

            def whc3(j):
                return whc_sb[:, j * H : (j + 1) * H].rearrange(
                    "p (kc m) -> p kc m", kc=KC
                )

            # ---- cold-start DMA issue trains ----
            # In-flight DMAs on one queue share descriptor-level round-robin
            # bandwidth, so each ring carries only same-criticality
            # transfers, every transfer is a contiguous per-partition run
            # (2-4KB lines), and the bulk j>=1 weight train is issued from
            # the ACT FIFO between sigmoids (compute-paced backpressure).
            # ACT ring cold: j0 weights + constants only. wz j0 is chunked
            # so the first matmul pairs start before the full tile lands.
            # ALL cold-start transfers ride ONE queue (sync) in strict need
            # order: with a single active queue there is no cross-queue
            # bandwidth competition, so the critical item is always among
            # the <=8 in-flight transfers and completes in need order.
            # (x16 b1, h16 j1 and the j>=1 weight train are issued from the
            # ACT FIFO after the first sigmoids — compute-paced.)
            nc.sync.dma_start(wzr_sb[:, 0:1024], Wg[:, 0:1024])  # wz j0 lo
            nc.sync.dma_start(x8_sb[:, 0:2048], x8[:, 0:2048])  # x8 b0 lo
            nc.sync.dma_start(wzr_sb[:, 1024:2048], Wg[:, 1024:2048])  # wz j0 hi
            nc.sync.dma_start(x8_sb[:, 2048:4096], x8[:, 2048:4096])  # x8 b0 hi
            nc.sync.dma_start(wzr_sb[:, 2048:4096], Wg[:, 2048:4096])  # wr j0
            nc.sync.dma_start(h8_sb[:, 0:4096], h8[:, 0:4096])  # h8 b0
            nc.sync.dma_start(bg_sb[:], bg[:])
            nc.sync.dma_start(x8_sb[:, 4096:8192], x8[:, 4096:8192])  # x8 b1
            nc.sync.dma_start(h8_sb[:, 4096:8192], h8[:, 4096:8192])  # h8 b1
            nc.sync.dma_start(whc_sb[:, 0:H], Whc[:, 0:H])  # whc j0
            nc.sync.dma_start(wc8_sb[:, 0:512], Wc8[:, 0:512])  # wc8 j0
            nc.sync.dma_start(wc_sb[:, 0:512], Wc[:, 0:512])  # wc j0
            nc.sync.dma_start(x16_sb[:, 0:1024], x16[:, 0:1024])  # x16 b0 lo
            nc.sync.dma_start(x16_sb[:, 1024:2048], x16[:, 1024:2048])  # x16 b0 hi
            nc.sync.dma_start(bhc_sb[:], bhc[:])
            nc.sync.dma_start(bc_sb[:], bc[:])

            def gated_dma(sb, dram, a, b, gate_src):
                # The scheduler hoists DMA issues ahead of compute, so every
                # bulk transfer gets a real WAW hazard: a 1-column copy from
                # a compute result into the DMA's destination region. The
                # issue then cannot run before that compute finished. All on
                # the GPSIMD FIFO, which carries no latency-critical work.
                nc.gpsimd.tensor_copy(sb[:, a : a + 1], gate_src)
                nc.gpsimd.dma_start(sb[:, a:b], dram[:, a:b])

            def load_weights(j, gate_src):
                gated_dma(wzr_sb, Wg, j * 4096, (j + 1) * 4096, gate_src)
                gated_dma(whc_sb, Whc, j * H, (j + 1) * H, gate_src)
                gated_dma(wc_sb, Wc, j * 512, (j + 1) * 512, gate_src)
                gated_dma(wc8_sb, Wc8, j * 512, (j + 1) * 512, gate_src)

            def gate_matmuls(psum, w3, hf, cs=range(KC)):
                # accumulate over [x;h]: 8 DoubleRow passes of K=256 each;
                # pair c<4 reads x8, c>=4 reads h8
                for c in cs:
                    src = x8_sb if c < KC // 2 else h8_sb
                    nc.tensor.matmul(
                        psum[:],
                        lhsT=w3[:, 2 * c : 2 * c + 2, :],
                        rhs=pair8(src, hf, (2 * c) % KC),
                        start=(c == 0),
                        stop=(c == KC - 1),
                        perf_mode=DR,
                    )

            def hc_matmuls(psum, j, hf):
                w3 = whc3(j)
                for c in range(KC // 2):
                    nc.tensor.matmul(
                        psum[:],
                        lhsT=w3[:, 2 * c : 2 * c + 2, :],
                        rhs=pair8(h8_sb, hf, 2 * c),
                        start=(c == 0),
                        stop=(c == KC // 2 - 1),
                        perf_mode=DR,
                    )

            def c_matmuls(psum, j, hf):
                # contraction kc0-3 in fp8 DoubleRow (via resident x8),
                # kc4-7 in fp16 — error-budget split measured at 1.77e-2
                # total vs the 2e-2 gate
                w83 = wc8_sb[:, j * 512 : (j + 1) * 512].rearrange(
                    "p (kc m) -> p kc m", kc=4
                )
                for c in range(2):
                    nc.tensor.matmul(
                        psum[:],
                        lhsT=w83[:, 2 * c : 2 * c + 2, :],
                        rhs=pair8(x8_sb, hf, 2 * c),
                        start=(c == 0),
                        stop=False,
                        perf_mode=DR,
                    )
                for k in range(4):
                    nc.tensor.matmul(
                        psum[:],
                        lhsT=wc_sb[:, (j * 4 + k) * P : (j * 4 + k + 1) * P],
                        rhs=x16_sb[:, hf * 2048 + k * 512 : hf * 2048 + k * 512 + 512],
                        start=False,
                        stop=(k == 3),
                    )

            def act_z(j, b0, pz):
                # z = sigmoid(pz/4096 + bg_z); zh = (z-1)*h  (fp16, 2x DVE)
                z = gpool.tile([P, 512], F16, tag="g")
                nc.scalar.activation(
                    z[:], pz[:], AF.Sigmoid,
                    bias=bg_sb[:, j : j + 1], scale=SCALE_INV,
                )
                zh = wpool.tile([P, 512], F16, tag="w16")
                nc.vector.scalar_tensor_tensor(
                    zh[:], z[:], 1.0, h16_sb[:, j * BL + b0 : j * BL + b0 + 512],
                    ALU.subtract, ALU.mult,
                )
                return z, zh

            def act_r(j, pr):
                r = gpool.tile([P, 512], F32, tag="gr")
                nc.scalar.activation(
                    r[:], pr[:], AF.Sigmoid,
                    bias=bg_sb[:, NJ + j : NJ + j + 1], scale=SCALE_INV,
                )
                return r

            def blend(j, b0, z, zh, r, ph, px):
                # cand = tanh((px + (ph + bhc~)*r)/4096 + bc);
                # out = z*cand - (z-1)*h
                rh = wpool.tile([P, 512], F32, tag="w32")
                nc.vector.scalar_tensor_tensor(
                    rh[:], ph[:], bhc_sb[:, j : j + 1], r[:], ALU.add, ALU.mult
                )
                s = wpool.tile([P, 512], F32, tag="w32")
                nc.vector.tensor_add(s[:], px[:], rh[:])
                cand = wpool.tile([P, 512], F16, tag="w16")
                nc.scalar.activation(
                    cand[:], s[:], AF.Tanh, bias=bc_sb[:, j : j + 1], scale=SCALE_INV
                )
                m = wpool.tile([P, 512], F16, tag="w16")
                nc.vector.tensor_mul(m[:], z[:], cand[:])
                o = wpool.tile([P, 512], F16, tag="w16")
                nc.vector.tensor_sub(o[:], m[:], zh[:])
                nc.sync.dma_start(
                    outT[:, j * BL + b0 : j * BL + b0 + 512], o[:]
                )

            # ---- j = 0: gates for both b-halves first (their fp8 operands
            # arrive first), candidate x-matmuls last (x16 arrives later).
            # Uses all 8 PSUM banks.
            pz0 = ppool.tile([P, 512], F32, tag="ps")
            pr0 = ppool.tile([P, 512], F32, tag="ps")
            # interleave z/r with DMA arrival: wz0, x8 b0, wr0, h8 b0
            gate_matmuls(pz0, wz3(0), 0, cs=range(0, 4))
            gate_matmuls(pr0, wr3(0), 0, cs=range(0, 4))
            gate_matmuls(pz0, wz3(0), 0, cs=range(4, 8))
            gate_matmuls(pr0, wr3(0), 0, cs=range(4, 8))
            # h16 j0 and the j1 weight train gate on the completed pz0 PSUM
            # via DVE copies (GPSIMD can't read PSUM; this fires ~1us before
            # the z0 sigmoid result exists)
            nc.vector.tensor_copy(h16_sb[:, 0:1], pz0[:, 0:1])
            nc.gpsimd.dma_start(h16_sb[:, 0:BL], h16[:, 0:BL])  # h16 j0
            for sb, dram, a, b in (
                (x16_sb, x16, 2048, 3072),  # x16 b1 lo
                (x16_sb, x16, 3072, 4096),  # x16 b1 hi
                (wzr_sb, Wg, 4096, 8192),
                (whc_sb, Whc, H, 2 * H),
                (wc_sb, Wc, 512, 1024),
                (wc8_sb, Wc8, 512, 1024),
            ):
                nc.vector.tensor_copy(sb[:, a : a + 1], pz0[:, 0:1])
                nc.gpsimd.dma_start(sb[:, a:b], dram[:, a:b])
            z0, zh0 = act_z(0, 0, pz0)
            zg0 = z0[:, 0:1]
            r0 = act_r(0, pr0)
            gated_dma(h16_sb, h16, BL, 2 * BL, zg0)  # h16 j1
            pz1 = ppool.tile([P, 512], F32, tag="ps")
            gate_matmuls(pz1, wz3(0), 1, cs=range(0, 4))
            pr1 = ppool.tile([P, 512], F32, tag="ps")
            gate_matmuls(pr1, wr3(0), 1, cs=range(0, 4))
            gate_matmuls(pz1, wz3(0), 1, cs=range(4, 8))
            gate_matmuls(pr1, wr3(0), 1, cs=range(4, 8))
            z1, zh1 = act_z(0, 512, pz1)
            r1 = act_r(0, pr1)
            ph0 = ppool.tile([P, 512], F32, tag="ps")
            hc_matmuls(ph0, 0, 0)
            ph1 = ppool.tile([P, 512], F32, tag="ps")
            hc_matmuls(ph1, 0, 1)
            px0 = ppool.tile([P, 512], F32, tag="ps")
            c_matmuls(px0, 0, 0)
            blend(0, 0, z0, zh0, r0, ph0, px0)
            px1 = ppool.tile([P, 512], F32, tag="ps")
            c_matmuls(px1, 0, 1)
            blend(0, 512, z1, zh1, r1, ph1, px1)

            # j2 stream gated on z1
            gated_dma(h16_sb, h16, 2 * BL, 3 * BL, z1[:, 0:1])  # h16 j2
            load_weights(2, z1[:, 0:1])

            # ---- steady state ----
            for j in range(1, NJ):
                for b in range(NB):
                    b0 = b * 512
                    if j == NJ - 1 and b == NB - 1:
                        break  # last tile handled below
                    pz = ppool.tile([P, 512], F32, tag="ps")
                    gate_matmuls(pz, wz3(j), b)
                    z, zh = act_z(j, b0, pz)
                    pr = ppool.tile([P, 512], F32, tag="ps")
                    gate_matmuls(pr, wr3(j), b)
                    r = act_r(j, pr)
                    if b == 0 and j + 2 < NJ:
                        gated_dma(
                            h16_sb, h16, (j + 2) * BL, (j + 3) * BL, z[:, 0:1]
                        )
                        load_weights(j + 2, z[:, 0:1])
                    ph = ppool.tile([P, 512], F32, tag="ps")
                    hc_matmuls(ph, j, b)
                    px = ppool.tile([P, 512], F32, tag="ps")
                    c_matmuls(px, j, b)
                    blend(j, b0, z, zh, r, ph, px)

            # ---- last tile: z-gate matmuls LAST, blend in 256-wide halves
            # so the post-matmul serial chain is short.
            j, b0 = NJ - 1, 512
            ph = ppool.tile([P, 512], F32, tag="ps")
            hc_matmuls(ph, j, 1)
            px = ppool.tile([P, 512], F32, tag="ps")
            c_matmuls(px, j, 1)
            pr = ppool.tile([P, 512], F32, tag="ps")
            gate_matmuls(pr, wr3(j), 1)
            pz = ppool.tile([P, 512], F32, tag="ps")
            gate_matmuls(pz, wz3(j), 1)
            r = act_r(j, pr)
            rh = wpool.tile([P, 512], F32, tag="w32")
            s = wpool.tile([P, 512], F32, tag="w32")
            cand = wpool.tile([P, 512], F16, tag="w16")
            z = gpool.tile([P, 512], F16, tag="g")
            zh = wpool.tile([P, 512], F16, tag="w16")
            m = wpool.tile([P, 512], F16, tag="w16")
            o = wpool.tile([P, 512], F16, tag="w16")
            H2 = 256
            for lo in (0, H2):
                nc.vector.scalar_tensor_tensor(
                    rh[:, lo : lo + H2], ph[:, lo : lo + H2],
                    bhc_sb[:, j : j + 1], r[:, lo : lo + H2], ALU.add, ALU.mult,
                )
            for lo in (0, H2):
                nc.vector.tensor_add(
                    s[:, lo : lo + H2], px[:, lo : lo + H2], rh[:, lo : lo + H2]
                )
            for lo in (0, H2):
                nc.scalar.activation(
                    cand[:, lo : lo + H2], s[:, lo : lo + H2], AF.Tanh,
                    bias=bc_sb[:, j : j + 1], scale=SCALE_INV,
                )
            for lo in (0, H2):
                nc.scalar.activation(
                    z[:, lo : lo + H2], pz[:, lo : lo + H2], AF.Sigmoid,
                    bias=bg_sb[:, j : j + 1], scale=SCALE_INV,
                )
            for lo in (0, H2):
                nc.vector.scalar_tensor_tensor(
                    zh[:, lo : lo + H2], z[:, lo : lo + H2], 1.0,
                    h16_sb[:, j * BL + b0 + lo : j * BL + b0 + lo + H2],
                    ALU.subtract, ALU.mult,
                )
            for lo in (0, H2):
                nc.vector.tensor_mul(
                    m[:, lo : lo + H2], z[:, lo : lo + H2], cand[:, lo : lo + H2]
                )
            for lo in (0, H2):
                nc.vector.tensor_sub(
                    o[:, lo : lo + H2], m[:, lo : lo + H2], zh[:, lo : lo + H2]
                )
            nc.sync.dma_start(outT[:, j * BL + b0 : j * BL + b0 + 512], o[:])

    nc.compile()
    return nc


def _pack_weights(W_ih, b_ih, W_hh, b_hh, W_c, b_c, W_hc, b_hc):
    f8 = ml_dtypes.float8_e4m3
    Wg_full = np.concatenate([W_ih, W_hh], axis=0)  # [2H, 2H] = [k, o]
    # [kc, p, g, j, jj] -> [p, j, g, kc, jj]
    WgH = np.ascontiguousarray(
        Wg_full.reshape(16, P, 2, NJ, P).transpose(1, 3, 2, 0, 4).reshape(P, NJ * 4096)
        * WSCALE
    ).astype(f8)
    WcH = np.ascontiguousarray(
        W_c[512:].reshape(4, P, NJ, P).transpose(1, 2, 0, 3).reshape(P, NJ * 512)
        * (ASCALE * WSCALE)
    ).astype(np.float16)
    Wc8H = np.ascontiguousarray(
        W_c[:512].reshape(4, P, NJ, P).transpose(1, 2, 0, 3).reshape(P, NJ * 512)
        * WSCALE
    ).astype(f8)
    WhcH = np.ascontiguousarray(
        W_hc.reshape(KC, P, NJ, P).transpose(1, 2, 0, 3).reshape(P, NJ * H) * WSCALE
    ).astype(f8)
    bgH = np.ascontiguousarray((b_ih + b_hh).reshape(16, P).T).astype(np.float32)
    bcH = np.ascontiguousarray(b_c.reshape(NJ, P).T).astype(np.float32)
    bhcH = np.ascontiguousarray(b_hc.reshape(NJ, P).T * (ASCALE * WSCALE)).astype(
        np.float32
    )
    return WgH, WcH, Wc8H, WhcH, bgH, bcH, bhcH


def _pack_acts(a, dtype, scale=1.0):
    # [BL, H] -> [p, hf*(KC*512) + kc*512 + b] with a[hf*512 + b, kc*128+p]
    out = a.T.reshape(KC, P, NB, 512).transpose(1, 2, 0, 3).reshape(P, KC * BL)
    if scale != 1.0:
        out = out * scale
    return np.ascontiguousarray(out).astype(dtype)


def _pack_x16(a):
    # kc4-7 only (the fp8-DoubleRow half of x@W_c reads x8 instead):
    # [p, hf*2048 + (kc-4)*512 + b]
    return np.ascontiguousarray(
        a.T[512:].reshape(4, P, NB, 512).transpose(1, 2, 0, 3).reshape(P, 4 * BL)
    ).astype(np.float16)


def _pack_h(a):
    # j-major residual layout: [p, j*BL + b] with a[b, j*128+p]
    return np.ascontiguousarray(
        a.T.reshape(NJ, P, BL).transpose(1, 0, 2).reshape(P, NJ * BL)
    ).astype(np.float16)


def kernel(input, hx, W_ih, b_ih, W_hh, b_hh, W_c, b_c, W_hc, b_hc):
    input = np.asarray(input, np.float32)
    hx = np.asarray(hx, np.float32)
    if "nc" not in _CACHE:
        _CACHE["nc"] = _build_program()
    nc = _CACHE["nc"]

    WgH, WcH, Wc8H, WhcH, bgH, bcH, bhcH = _pack_weights(
        np.asarray(W_ih, np.float32), np.asarray(b_ih, np.float32),
        np.asarray(W_hh, np.float32), np.asarray(b_hh, np.float32),
        np.asarray(W_c, np.float32), np.asarray(b_c, np.float32),
        np.asarray(W_hc, np.float32), np.asarray(b_hc, np.float32),
    )

    f8 = ml_dtypes.float8_e4m3
    in_maps = []
    for i in range(N_CORES):
        xs = input[i * BL : (i + 1) * BL]
        hs = hx[i * BL : (i + 1) * BL]
        in_maps.append(
            {
                "x8": _pack_acts(xs, f8, ASCALE),
                "h8": _pack_acts(hs, f8, ASCALE),
                "x16": _pack_x16(xs),
                "h16": _pack_h(hs),
                "Wg": WgH,
                "Wc": WcH,
                "Wc8": Wc8H,
                "Whc": WhcH,
                "bg": bgH,
                "bc": bcH,
                "bhc": bhcH,
            }
        )

    res = run_bass_kernel_spmd(nc, in_maps, core_ids=list(range(N_CORES)))
    out = np.empty((B, H), np.float32)
    for i, r in enumerate(res.results):
        o = (
            np.asarray(r["outT"], dtype=np.float32)
            .reshape(P, NJ, BL)
            .transpose(2, 1, 0)
            .reshape(BL, H)
        )
        out[i * BL : (i + 1) * BL] = o
    return out


# revision 53
# speedup vs baseline: 1.0560x; 1.0197x over previous
"""GRU-cell-variant kernel for Trainium2, data-parallel over batch on 8 cores.

Reference (per batch row b, hidden size H=1024):
    gates = sigmoid(x @ W_ih + b_ih + h @ W_hh + b_hh)   # [B, 2H]
    z, r  = gates[:, :H], gates[:, H:]
    cand  = tanh(x @ W_c + b_c + r * (h @ W_hc + b_hc))
    out   = (1 - z) * h + z * cand

Design:
  - 8-way batch shard (1024 rows/core), weights replicated. No collectives.
  - Everything on-chip is computed TRANSPOSED: out.T[o, b]. Weight tiles
    [k, o] load naturally as the stationary operand, host-pre-transposed
    x.T / h.T serve as the moving operand, and all biases are per-partition
    (free bias-add on the ACT engine).
  - Mixed fp8/fp16 matmuls: the z/r gate matmuls, h@W_hc, and half of
    x@W_c's contraction run as fp8-e4m3 DoubleRow (2 contraction chunks
    per PE pass -> 2x the fp16 matmul rate, 216ns per K=256xN=512 pass
    measured); the other half of x@W_c stays fp16 because its
    quantization error feeds tanh unattenuated. Measured L2 error:
    all-fp8 2.05e-2 (fails), this mix 1.77e-2 vs the 2e-2 gate.
  - fp8 operands are pre-scaled on the host (x,h by 2^4; W by 2^8) to stay
    clear of e4m3 subnormals; the combined 2^-12 descale folds into the
    scale parameter of the existing sigmoid/tanh activation ops. The fp16
    W_c half is pre-scaled by 2^12 so all candidate partial sums share one
    scale. fp32 PSUM accumulation throughout; h-residual path in fp16.
  - All weights are SBUF-resident and streamed in once. DMA discipline
    (HWDGE serves in-flight DMAs round-robin per descriptor, and the Tile
    scheduler hoists DMA issues ahead of compute): (1) every cold-start-
    critical transfer rides ONE queue (sync) in strict need order with
    nothing else on it; (2) every bulk transfer (x16 b1, h16 j1+, the
    j>=1 weight train) is issued from the compute-free GPSIMD FIFO behind
    an explicit WAW gate -- a 1-column DVE/GPSIMD copy from a compute
    result into the DMA's destination -- so it cannot start before the
    pipeline actually needs it; (3) output stores ride the sync queue,
    idle after cold start. ACT runs activations only.
  - Elementwise blend runs in fp16 where PSUM isn't involved (2x DVE),
    output is stored fp16 and upcast on the host. The last tile computes
    its z-gate matmul LAST and blends in 256-wide halves so the serial
    post-matmul chain is ~2.7us instead of ~10us.
"""

import numpy as np
import ml_dtypes

import concourse.bass as bass
import concourse.mybir as mybir
import concourse.tile as tile
from concourse import bacc
from concourse.bass_utils import run_bass_kernel_spmd

N_CORES = 8
B = 8192
H = 1024
BL = B // N_CORES  # batch rows per core
P = 128
KC = H // P  # 8 contraction chunks of 128 per 1024-wide operand
NJ = H // P  # 8 hidden-dim tiles
NB = BL // 512  # 2 moving halves of 512 batch columns

F8 = mybir.dt.float8e4
F16 = mybir.dt.float16
F32 = mybir.dt.float32
AF = mybir.ActivationFunctionType
ALU = mybir.AluOpType
DR = mybir.MatmulPerfMode.DoubleRow

ASCALE = 16.0  # activation fp8 pre-scale
WSCALE = 256.0  # weight fp8 pre-scale
SCALE_INV = 1.0 / (ASCALE * WSCALE)  # descale folded into ACT ops

_CACHE = {}


def _build_program():
    nc = bacc.Bacc(
        "TRN2",
        target_bir_lowering=False,
        debug=False,
        enable_asserts=False,
        num_devices=N_CORES,
    )

    # DRAM inputs, already packed on the host into SBUF-friendly layouts.
    # x8/h8:  [p, hf*4096 + kc*512 + b] = x[hf*512 + b, kc*128 + p] * 16
    #         (fp8 e4m3, batch-half-major so every cold DMA is a contiguous
    #          per-partition run with >=2KB lines — small-line strided DMAs
    #          get starved by the round-robin descriptor service when bulk
    #          weight DMAs share the queue)
    # x16:    same layout, unscaled fp16 (W_c matmul operand)
    # h16:    [p, j*BL + b] = h[b, j*128 + p]  fp16 (residual path)
    # Wg:     [p, j*4096 + g*2048 + kc*128 + jj]
    #           = concat([W_ih, W_hh])[kc*128+p, g*1024 + j*128 + jj] * 256
    #         (fp8; g=0 -> z gate, g=1 -> r gate; per-j single contiguous DMA)
    # Whc:    [p, j*1024 + kc*128 + jj] = W_hc[kc*128+p, j*128+jj]*256  (fp8)
    # Wc:     same layout, W_c * 4096  (fp16)
    # bg:     [p, t] = (b_ih+b_hh)[t*128+p]; bc analogous; bhc pre-scaled 4096.
    x8 = nc.dram_tensor("x8", [P, KC * BL], F8, kind="ExternalInput").ap()
    h8 = nc.dram_tensor("h8", [P, KC * BL], F8, kind="ExternalInput").ap()
    x16 = nc.dram_tensor("x16", [P, 4 * BL], F16, kind="ExternalInput").ap()
    h16 = nc.dram_tensor("h16", [P, NJ * BL], F16, kind="ExternalInput").ap()
    Wg = nc.dram_tensor("Wg", [P, NJ * 4096], F8, kind="ExternalInput").ap()
    Wc = nc.dram_tensor("Wc", [P, NJ * 512], F16, kind="ExternalInput").ap()
    Wc8 = nc.dram_tensor("Wc8", [P, NJ * 512], F8, kind="ExternalInput").ap()
    Whc = nc.dram_tensor("Whc", [P, NJ * H], F8, kind="ExternalInput").ap()
    bg = nc.dram_tensor("bg", [P, 16], F32, kind="ExternalInput").ap()
    bc = nc.dram_tensor("bc", [P, NJ], F32, kind="ExternalInput").ap()
    bhc = nc.dram_tensor("bhc", [P, NJ], F32, kind="ExternalInput").ap()
    outT = nc.dram_tensor("outT", [P, NJ * BL], F16, kind="ExternalOutput").ap()

    with tile.TileContext(nc) as tc:
        with (
            tc.tile_pool(name="const", bufs=1) as cpool,
            tc.tile_pool(name="psum", bufs=8, space="PSUM") as ppool,
            tc.tile_pool(name="gates", bufs=6) as gpool,
            tc.tile_pool(name="work", bufs=12) as wpool,
        ):
            bg_sb = cpool.tile([P, 16], F32, tag="bg")
            bc_sb = cpool.tile([P, NJ], F32, tag="bc")
            bhc_sb = cpool.tile([P, NJ], F32, tag="bhc")

            # Resident activations and weights.
            x8_sb = cpool.tile([P, KC * BL], F8, tag="x8")
            h8_sb = cpool.tile([P, KC * BL], F8, tag="h8")
            x16_sb = cpool.tile([P, 4 * BL], F16, tag="x16")
            h16_sb = cpool.tile([P, NJ * BL], F16, tag="h16")
            wzr_sb = cpool.tile([P, NJ * 4096], F8, tag="wzr")
            whc_sb = cpool.tile([P, NJ * H], F8, tag="whc")
            wc_sb = cpool.tile([P, NJ * 512], F16, tag="wc")
            wc8_sb = cpool.tile([P, NJ * 512], F8, tag="wc8")

            def pair8(sb, hf, k0):
                # [p, 2, 512] DoubleRow rhs view of contraction pair (k0, k0+1)
                off = hf * 4096 + k0 * 512
                return sb[:, off : off + 1024].rearrange("p (k b) -> p k b", k=2)

            def wz3(j):
                return wzr_sb[:, j * 4096 : j * 4096 + 2048].rearrange(
                    "p (kc m) -> p kc m", kc=16
                )

            def wr3(j):
                return wzr_sb[:, j * 4096 + 2048 : (j + 1) * 4096].rearrange(
                    "p (kc m) -> p kc m", kc=16
                )

            def whc3(j):
                return whc_sb[:, j * H : (j + 1) * H].rearrange(
                    "p (kc m) -> p kc m", kc=KC
                )

            # ---- cold-start DMA train ----
            # ALL cold-start transfers ride ONE queue (sync) in strict need
            # order: HWDGE serves in-flight DMAs round-robin per descriptor,
            # so a single active queue means the critical item is always
            # among the <=8 in-flight transfers and completes in need
            # order. Chunk sizes are deliberately coarse — finer splits add
            # co-residents to the window and dilute every item's share.
            # (x16 b1, h16 j1+ and the j>=1 weight train are WAW-gated on
            # compute below; outputs reuse this queue once it idles.)
            nc.sync.dma_start(wzr_sb[:, 0:1024], Wg[:, 0:1024])  # wz j0 lo
            nc.sync.dma_start(x8_sb[:, 0:2048], x8[:, 0:2048])  # x8 b0 lo
            nc.sync.dma_start(wzr_sb[:, 1024:2048], Wg[:, 1024:2048])  # wz j0 hi
            nc.sync.dma_start(x8_sb[:, 2048:4096], x8[:, 2048:4096])  # x8 b0 hi
            nc.sync.dma_start(wzr_sb[:, 2048:4096], Wg[:, 2048:4096])  # wr j0
            nc.sync.dma_start(h8_sb[:, 0:4096], h8[:, 0:4096])  # h8 b0
            nc.sync.dma_start(bg_sb[:], bg[:])
            nc.sync.dma_start(x8_sb[:, 4096:8192], x8[:, 4096:8192])  # x8 b1
            nc.sync.dma_start(h8_sb[:, 4096:8192], h8[:, 4096:8192])  # h8 b1
            nc.sync.dma_start(whc_sb[:, 0:H], Whc[:, 0:H])  # whc j0
            nc.sync.dma_start(wc8_sb[:, 0:512], Wc8[:, 0:512])  # wc8 j0
            nc.sync.dma_start(wc_sb[:, 0:512], Wc[:, 0:512])  # wc j0
            nc.sync.dma_start(x16_sb[:, 0:1024], x16[:, 0:1024])  # x16 b0 lo
            nc.sync.dma_start(x16_sb[:, 1024:2048], x16[:, 1024:2048])  # x16 b0 hi
            nc.sync.dma_start(bhc_sb[:], bhc[:])
            nc.sync.dma_start(bc_sb[:], bc[:])

            def gated_dma(sb, dram, a, b, gate_src):
                # The scheduler hoists DMA issues ahead of compute, so every
                # bulk transfer gets a real WAW hazard: a 1-column copy from
                # a compute result into the DMA's destination region. The
                # issue then cannot run before that compute finished. All on
                # the GPSIMD FIFO, which carries no latency-critical work.
                nc.gpsimd.tensor_copy(sb[:, a : a + 1], gate_src)
                nc.gpsimd.dma_start(sb[:, a:b], dram[:, a:b])

            def load_weights(j, gate_src):
                gated_dma(wzr_sb, Wg, j * 4096, (j + 1) * 4096, gate_src)
                gated_dma(whc_sb, Whc, j * H, (j + 1) * H, gate_src)
                gated_dma(wc_sb, Wc, j * 512, (j + 1) * 512, gate_src)
                gated_dma(wc8_sb, Wc8, j * 512, (j + 1) * 512, gate_src)

            def gate_matmuls(psum, w3, hf, cs=range(KC)):
                # accumulate over [x;h]: 8 DoubleRow passes of K=256 each;
                # pair c<4 reads x8, c>=4 reads h8
                for c in cs:
                    src = x8_sb if c < KC // 2 else h8_sb
                    nc.tensor.matmul(
                        psum[:],
                        lhsT=w3[:, 2 * c : 2 * c + 2, :],
                        rhs=pair8(src, hf, (2 * c) % KC),
                        start=(c == 0),
                        stop=(c == KC - 1),
                        perf_mode=DR,
                    )

            def hc_matmuls(psum, j, hf):
                w3 = whc3(j)
                for c in range(KC // 2):
                    nc.tensor.matmul(
                        psum[:],
                        lhsT=w3[:, 2 * c : 2 * c + 2, :],
                        rhs=pair8(h8_sb, hf, 2 * c),
                        start=(c == 0),
                        stop=(c == KC // 2 - 1),
                        perf_mode=DR,
                    )

            def c_matmuls(psum, j, hf):
                # contraction kc0-3 in fp8 DoubleRow (via resident x8),
                # kc4-7 in fp16 — error-budget split measured at 1.77e-2
                # total vs the 2e-2 gate
                w83 = wc8_sb[:, j * 512 : (j + 1) * 512].rearrange(
                    "p (kc m) -> p kc m", kc=4
                )
                for c in range(2):
                    nc.tensor.matmul(
                        psum[:],
                        lhsT=w83[:, 2 * c : 2 * c + 2, :],
                        rhs=pair8(x8_sb, hf, 2 * c),
                        start=(c == 0),
                        stop=False,
                        perf_mode=DR,
                    )
                for k in range(4):
                    nc.tensor.matmul(
                        psum[:],
                        lhsT=wc_sb[:, (j * 4 + k) * P : (j * 4 + k + 1) * P],
                        rhs=x16_sb[:, hf * 2048 + k * 512 : hf * 2048 + k * 512 + 512],
                        start=False,
                        stop=(k == 3),
                    )

            def act_z(j, b0, pz):
                # z = sigmoid(pz/4096 + bg_z); zh = (z-1)*h  (fp16, 2x DVE)
                z = gpool.tile([P, 512], F16, tag="g")
                nc.scalar.activation(
                    z[:], pz[:], AF.Sigmoid,
                    bias=bg_sb[:, j : j + 1], scale=SCALE_INV,
                )
                zh = wpool.tile([P, 512], F16, tag="w16")
                nc.vector.scalar_tensor_tensor(
                    zh[:], z[:], 1.0, h16_sb[:, j * BL + b0 : j * BL + b0 + 512],
                    ALU.subtract, ALU.mult,
                )
                return z, zh

            def act_r(j, pr):
                r = gpool.tile([P, 512], F32, tag="gr")
                nc.scalar.activation(
                    r[:], pr[:], AF.Sigmoid,
                    bias=bg_sb[:, NJ + j : NJ + j + 1], scale=SCALE_INV,
                )
                return r

            def blend(j, b0, z, zh, r, ph, px):
                # cand = tanh((px + (ph + bhc~)*r)/4096 + bc);
                # out = z*cand - (z-1)*h
                rh = wpool.tile([P, 512], F32, tag="w32")
                nc.vector.scalar_tensor_tensor(
                    rh[:], ph[:], bhc_sb[:, j : j + 1], r[:], ALU.add, ALU.mult
                )
                s = wpool.tile([P, 512], F32, tag="w32")
                nc.vector.tensor_add(s[:], px[:], rh[:])
                cand = wpool.tile([P, 512], F16, tag="w16")
                nc.scalar.activation(
                    cand[:], s[:], AF.Tanh, bias=bc_sb[:, j : j + 1], scale=SCALE_INV
                )
                m = wpool.tile([P, 512], F16, tag="w16")
                nc.vector.tensor_mul(m[:], z[:], cand[:])
                o = wpool.tile([P, 512], F16, tag="w16")
                nc.vector.tensor_sub(o[:], m[:], zh[:])
                nc.sync.dma_start(
                    outT[:, j * BL + b0 : j * BL + b0 + 512], o[:]
                )

            # ---- j = 0: gates for both b-halves first (their fp8 operands
            # arrive first), candidate x-matmuls last (x16 arrives later).
            # Uses all 8 PSUM banks.
            pz0 = ppool.tile([P, 512], F32, tag="ps")
            pr0 = ppool.tile([P, 512], F32, tag="ps")
            # interleave z/r with DMA arrival: wz0, x8 b0, wr0, h8 b0
            gate_matmuls(pz0, wz3(0), 0, cs=range(0, 4))
            gate_matmuls(pr0, wr3(0), 0, cs=range(0, 4))
            gate_matmuls(pz0, wz3(0), 0, cs=range(4, 8))
            gate_matmuls(pr0, wr3(0), 0, cs=range(4, 8))
            # h16 j0 and the j1 weight train gate on the completed pz0 PSUM
            # via DVE copies (GPSIMD can't read PSUM; this fires ~1us before
            # the z0 sigmoid result exists)
            nc.vector.tensor_copy(h16_sb[:, 0:1], pz0[:, 0:1])
            nc.gpsimd.dma_start(h16_sb[:, 0:BL], h16[:, 0:BL])  # h16 j0
            for sb, dram, a, b in (
                (x16_sb, x16, 2048, 3072),  # x16 b1 lo
                (x16_sb, x16, 3072, 4096),  # x16 b1 hi
                (wzr_sb, Wg, 4096, 8192),
                (whc_sb, Whc, H, 2 * H),
                (wc_sb, Wc, 512, 1024),
                (wc8_sb, Wc8, 512, 1024),
            ):
                nc.vector.tensor_copy(sb[:, a : a + 1], pz0[:, 0:1])
                nc.gpsimd.dma_start(sb[:, a:b], dram[:, a:b])
            z0, zh0 = act_z(0, 0, pz0)
            zg0 = z0[:, 0:1]
            r0 = act_r(0, pr0)
            gated_dma(h16_sb, h16, BL, 2 * BL, zg0)  # h16 j1
            pz1 = ppool.tile([P, 512], F32, tag="ps")
            gate_matmuls(pz1, wz3(0), 1, cs=range(0, 4))
            pr1 = ppool.tile([P, 512], F32, tag="ps")
            gate_matmuls(pr1, wr3(0), 1, cs=range(0, 4))
            gate_matmuls(pz1, wz3(0), 1, cs=range(4, 8))
            gate_matmuls(pr1, wr3(0), 1, cs=range(4, 8))
            z1, zh1 = act_z(0, 512, pz1)
            r1 = act_r(0, pr1)
            ph0 = ppool.tile([P, 512], F32, tag="ps")
            hc_matmuls(ph0, 0, 0)
            ph1 = ppool.tile([P, 512], F32, tag="ps")
            hc_matmuls(ph1, 0, 1)
            px0 = ppool.tile([P, 512], F32, tag="ps")
            c_matmuls(px0, 0, 0)
            blend(0, 0, z0, zh0, r0, ph0, px0)
            px1 = ppool.tile([P, 512], F32, tag="ps")
            c_matmuls(px1, 0, 1)
            blend(0, 512, z1, zh1, r1, ph1, px1)

            # j2 stream gated on z1
            gated_dma(h16_sb, h16, 2 * BL, 3 * BL, z1[:, 0:1])  # h16 j2
            load_weights(2, z1[:, 0:1])

            # ---- steady state ----
            for j in range(1, NJ):
                for b in range(NB):
                    b0 = b * 512
                    if j == NJ - 1 and b == NB - 1:
                        break  # last tile handled below
                    pz = ppool.tile([P, 512], F32, tag="ps")
                    gate_matmuls(pz, wz3(j), b)
                    z, zh = act_z(j, b0, pz)
                    pr = ppool.tile([P, 512], F32, tag="ps")
                    gate_matmuls(pr, wr3(j), b)
                    r = act_r(j, pr)
                    if b == 0 and j + 2 < NJ:
                        gated_dma(
                            h16_sb, h16, (j + 2) * BL, (j + 3) * BL, z[:, 0:1]
                        )
                        load_weights(j + 2, z[:, 0:1])
                    ph = ppool.tile([P, 512], F32, tag="ps")
                    hc_matmuls(ph, j, b)
                    px = ppool.tile([P, 512], F32, tag="ps")
                    c_matmuls(px, j, b)
                    blend(j, b0, z, zh, r, ph, px)

            # ---- last tile: z-gate matmuls LAST, blend in 256-wide halves
            # so the post-matmul serial chain is short.
            j, b0 = NJ - 1, 512
            ph = ppool.tile([P, 512], F32, tag="ps")
            hc_matmuls(ph, j, 1)
            px = ppool.tile([P, 512], F32, tag="ps")
            c_matmuls(px, j, 1)
            pr = ppool.tile([P, 512], F32, tag="ps")
            gate_matmuls(pr, wr3(j), 1)
            pz = ppool.tile([P, 512], F32, tag="ps")
            gate_matmuls(pz, wz3(j), 1)
            r = act_r(j, pr)
            rh = wpool.tile([P, 512], F32, tag="w32")
            s = wpool.tile([P, 512], F32, tag="w32")
            cand = wpool.tile([P, 512], F16, tag="w16")
            z = gpool.tile([P, 512], F16, tag="g")
            zh = wpool.tile([P, 512], F16, tag="w16")
            m = wpool.tile([P, 512], F16, tag="w16")
            o = wpool.tile([P, 512], F16, tag="w16")
            H2 = 256
            for lo in (0, H2):
                nc.vector.scalar_tensor_tensor(
                    rh[:, lo : lo + H2], ph[:, lo : lo + H2],
                    bhc_sb[:, j : j + 1], r[:, lo : lo + H2], ALU.add, ALU.mult,
                )
            for lo in (0, H2):
                nc.vector.tensor_add(
                    s[:, lo : lo + H2], px[:, lo : lo + H2], rh[:, lo : lo + H2]
                )
            for lo in (0, H2):
                nc.scalar.activation(
                    cand[:, lo : lo + H2], s[:, lo : lo + H2], AF.Tanh,
                    bias=bc_sb[:, j : j + 1], scale=SCALE_INV,
                )
            for lo in (0, H2):
                nc.scalar.activation(
                    z[:, lo : lo + H2], pz[:, lo : lo + H2], AF.Sigmoid,
                    bias=bg_sb[:, j : j + 1], scale=SCALE_INV,
                )
            for lo in (0, H2):
                nc.vector.scalar_tensor_tensor(
                    zh[:, lo : lo + H2], z[:, lo : lo + H2], 1.0,
                    h16_sb[:, j * BL + b0 + lo : j * BL + b0 + lo + H2],
                    ALU.subtract, ALU.mult,
                )
            for lo in (0, H2):
                nc.vector.tensor_mul(
                    m[:, lo : lo + H2], z[:, lo : lo + H2], cand[:, lo : lo + H2]
                )
            for lo in (0, H2):
                nc.vector.tensor_sub(
                    o[:, lo : lo + H2], m[:, lo : lo + H2], zh[:, lo : lo + H2]
                )
            nc.sync.dma_start(outT[:, j * BL + b0 : j * BL + b0 + 512], o[:])

    nc.compile()
    return nc


def _pack_weights(W_ih, b_ih, W_hh, b_hh, W_c, b_c, W_hc, b_hc):
    f8 = ml_dtypes.float8_e4m3
    Wg_full = np.concatenate([W_ih, W_hh], axis=0)  # [2H, 2H] = [k, o]
    # [kc, p, g, j, jj] -> [p, j, g, kc, jj]
    WgH = np.ascontiguousarray(
        Wg_full.reshape(16, P, 2, NJ, P).transpose(1, 3, 2, 0, 4).reshape(P, NJ * 4096)
        * WSCALE
    ).astype(f8)
    WcH = np.ascontiguousarray(
        W_c[512:].reshape(4, P, NJ, P).transpose(1, 2, 0, 3).reshape(P, NJ * 512)
        * (ASCALE * WSCALE)
    ).astype(np.float16)
    Wc8H = np.ascontiguousarray(
        W_c[:512].reshape(4, P, NJ, P).transpose(1, 2, 0, 3).reshape(P, NJ * 512)
        * WSCALE
    ).astype(f8)
    WhcH = np.ascontiguousarray(
        W_hc.reshape(KC, P, NJ, P).transpose(1, 2, 0, 3).reshape(P, NJ * H) * WSCALE
    ).astype(f8)
    bgH = np.ascontiguousarray((b_ih + b_hh).reshape(16, P).T).astype(np.float32)
    bcH = np.ascontiguousarray(b_c.reshape(NJ, P).T).astype(np.float32)
    bhcH = np.ascontiguousarray(b_hc.reshape(NJ, P).T * (ASCALE * WSCALE)).astype(
        np.float32
    )
    return WgH, WcH, Wc8H, WhcH, bgH, bcH, bhcH


def _pack_acts(a, dtype, scale=1.0):
    # [BL, H] -> [p, hf*(KC*512) + kc*512 + b] with a[hf*512 + b, kc*128+p]
    out = a.T.reshape(KC, P, NB, 512).transpose(1, 2, 0, 3).reshape(P, KC * BL)
    if scale != 1.0:
        out = out * scale
    return np.ascontiguousarray(out).astype(dtype)


def _pack_x16(a):
    # kc4-7 only (the fp8-DoubleRow half of x@W_c reads x8 instead):
    # [p, hf*2048 + (kc-4)*512 + b]
    return np.ascontiguousarray(
        a.T[512:].reshape(4, P, NB, 512).transpose(1, 2, 0, 3).reshape(P, 4 * BL)
    ).astype(np.float16)


def _pack_h(a):
    # j-major residual layout: [p, j*BL + b] with a[b, j*128+p]
    return np.ascontiguousarray(
        a.T.reshape(NJ, P, BL).transpose(1, 0, 2).reshape(P, NJ * BL)
    ).astype(np.float16)


def kernel(input, hx, W_ih, b_ih, W_hh, b_hh, W_c, b_c, W_hc, b_hc):
    input = np.asarray(input, np.float32)
    hx = np.asarray(hx, np.float32)
    if "nc" not in _CACHE:
        _CACHE["nc"] = _build_program()
    nc = _CACHE["nc"]

    WgH, WcH, Wc8H, WhcH, bgH, bcH, bhcH = _pack_weights(
        np.asarray(W_ih, np.float32), np.asarray(b_ih, np.float32),
        np.asarray(W_hh, np.float32), np.asarray(b_hh, np.float32),
        np.asarray(W_c, np.float32), np.asarray(b_c, np.float32),
        np.asarray(W_hc, np.float32), np.asarray(b_hc, np.float32),
    )

    f8 = ml_dtypes.float8_e4m3
    in_maps = []
    for i in range(N_CORES):
        xs = input[i * BL : (i + 1) * BL]
        hs = hx[i * BL : (i + 1) * BL]
        in_maps.append(
            {
                "x8": _pack_acts(xs, f8, ASCALE),
                "h8": _pack_acts(hs, f8, ASCALE),
                "x16": _pack_x16(xs),
                "h16": _pack_h(hs),
                "Wg": WgH,
                "Wc": WcH,
                "Wc8": Wc8H,
                "Whc": WhcH,
                "bg": bgH,
                "bc": bcH,
                "bhc": bhcH,
            }
        )

    res = run_bass_kernel_spmd(nc, in_maps, core_ids=list(range(N_CORES)))
    out = np.empty((B, H), np.float32)
    for i, r in enumerate(res.results):
        o = (
            np.asarray(r["outT"], dtype=np.float32)
            .reshape(P, NJ, BL)
            .transpose(2, 1, 0)
            .reshape(BL, H)
        )
        out[i * BL : (i + 1) * BL] = o
    return out
